# revision 1
# baseline (speedup 1.0000x reference)
"""Trainium2 Bass kernel for nn_BasicDeconvolutionBlock.

Reference computation (see problem statement):
    gathered = feats[in_map]                         # [K, M, Cin]
    contrib  = einsum('kmc,kcd->kmd', gathered, W)   # [K, M, Cout]
    out      = zeros([n_out, Cout]).at[out_map].add(contrib)
    y        = relu(batchnorm(out))                  # batch stats over n_out rows

Strategy (8 NeuronCores, SPMD):
  - Host routes each (k, m) pair to the core owning its output row
    (row blocks of n_out/8).  Per core ~169k pairs.
  - Gather: feats pre-cast to bf16, padded to 128 channels (256B rows).
    dma_gather(transpose=True) produces a CHANNEL-MAJOR SBUF slab
    G[128ch, slots] directly.  int16 gather indices -> feats is split in
    chunks of 32768 rows; pairs are grouped by (chunk, k), groups padded
    to a multiple of 128 slots.
  - GEMM: per 128-slot tile, matmul(lhsT=G_tile[128ch,128slots] (stationary),
    rhs=Wpad[k][128ch,64]) -> PSUM contrib[128slots, 64] fp32 (m-major,
    no transposes anywhere).
  - Scatter: DVE copies PSUM->SBUF slab, then gpsimd dma_scatter_add
    (CCE-add, int16 idx) accumulates rows into one of two HBM accumulator
    banks (cycled by round parity so chains overlap).  Duplicate rows race
    in hardware, so a host-side occurrence-round split guarantees unique
    rows per call; same-bank calls serialize via Tile WAW deps.  SWDGE
    calls are capped at 896 indices (the Q7 ucode descriptor-ring limit;
    larger calls hard-wedge the device).
  - BN: ones-matmul row sums + sum of squares, [2,64] AllReduce across
    the 8 cores, normalize + ReLU on chip, output shard [rows,64] fp32.
"""

import os
import sys

import numpy as np

sys.path.insert(0, "/opt/trn_rl_repo")

import ml_dtypes  # noqa: E402

from concourse import bacc, bass, mybir  # noqa: E402
import concourse.tile as tile  # noqa: E402

BN_EPS = 1e-5
CHUNK = 32768  # int16 gather index range per feats chunk
SEG_SLOTS = 896  # max slots per SWDGE call; 1024+ wedges the device (Q7 ucode descriptor-ring limit, verified empirically)
F32 = mybir.dt.float32
BF16 = mybir.dt.bfloat16
I16 = mybir.dt.int16
I32 = mybir.dt.int32


def _roundup(x, m):
    return (x + m - 1) // m * m


def _route(in_map, out_map, n_out, n_cores, dup_safe, expand=1):
    """Host-side routing. Returns compile-time plan + per-core packed arrays.

    Slot stream per core: for r in rounds, for c in chunks, for k in K:
    group (r,c,k) padded to a multiple of 128 slots.  If dup_safe, a single
    round (r=0) is used (occurrence splitting disabled).

    expand=E spreads a row's duplicate contributions over E contiguous
    accumulator banks (phys row = (occ%E)*acc_rows + row, round = occ//E),
    halving/quartering the round count; the kernel folds banks before BN.
    """
    K, M = in_map.shape
    rows_per_core = n_out // n_cores
    assert rows_per_core * n_cores == n_out
    acc_rows = _roundup(rows_per_core, 128)
    nchunk = _roundup(int(in_map.max()) + 1, CHUNK) // CHUNK

    k_idx = np.repeat(np.arange(K, dtype=np.int32), M)
    in_flat = in_map.ravel().astype(np.int64)
    out_flat = out_map.ravel().astype(np.int64)
    core = out_flat // rows_per_core
    row_local = (out_flat - core * rows_per_core).astype(np.int32)
    chunk = (in_flat // CHUNK).astype(np.int32)
    idx_local = (in_flat - chunk.astype(np.int64) * CHUNK).astype(np.int32)

    per_core = []
    max_round = 1
    for c in range(n_cores):
        sel = np.nonzero(core == c)[0]
        rows_c = row_local[sel]
        if dup_safe:
            rnd = np.zeros(len(sel), dtype=np.int32)
            prow = rows_c.astype(np.int32)
        else:
            order = np.argsort(rows_c, kind="stable")
            sr = rows_c[order]
            n = len(sr)
            first = np.ones(n, dtype=bool)
            first[1:] = sr[1:] != sr[:-1]
            grp_start = np.maximum.accumulate(np.where(first, np.arange(n), 0))
            occ_sorted = np.arange(n) - grp_start
            occ = np.empty(n, dtype=np.int64)
            occ[order] = occ_sorted
            rnd = (occ // expand).astype(np.int32)
            prow = (rows_c + (occ % expand) * acc_rows).astype(np.int32)
            max_round = max(max_round, int(rnd.max()) + 1 if n else 1)
        per_core.append(
            dict(rnd=rnd, chunk=chunk[sel], k=k_idx[sel],
                 idx=idx_local[sel], row=prow)
        )

    R = max_round
    # group counts [R, nchunk, K] per core -> shared caps
    counts = np.zeros((n_cores, R, nchunk, K), dtype=np.int64)
    for c in range(n_cores):
        p = per_core[c]
        np.add.at(counts[c], (p["rnd"], p["chunk"], p["k"]), 1)
    caps = (np.ceil(counts.max(axis=0) / 128).astype(np.int64) * 128)  # [R,nchunk,K]

    # segments: contiguous runs of (r,c,k) group pieces, same (r,c),
    # <= SEG_SLOTS per segment (SWDGE per-instruction descriptor limit).
    # Groups larger than SEG_SLOTS are split across segments.
    segments = []  # dicts: r, c, slot0 (global), nslots, groups=[(k, len, off_in_seg)]
    group_slot0 = {}  # (r,c,k) -> global slot of the group's first slot
    slot0 = 0
    for r in range(R):
        for c in range(nchunk):
            cur = None
            for k in range(K):
                cap = int(caps[r, c, k])
                if cap == 0:
                    continue
                group_slot0[(r, c, k)] = slot0 + (cur["nslots"] if cur else 0)
                rem = cap
                while rem > 0:
                    if cur is None:
                        cur = dict(r=r, c=c, slot0=slot0, nslots=0, groups=[])
                    take = min(SEG_SLOTS - cur["nslots"], rem)
                    if take == 0:
                        segments.append(cur)
                        slot0 += cur["nslots"]
                        cur = None
                        continue
                    cur["groups"].append((k, take, cur["nslots"]))
                    cur["nslots"] += take
                    rem -= take
            if cur is not None:
                segments.append(cur)
                slot0 += cur["nslots"]
                cur = None
    total_slots = slot0

    dump_row = expand * acc_rows  # rows beyond the banks are the dump zone
    acc_total = expand * acc_rows + 128

    # pack per-core gather idx and scatter idx (both int16, wrapped 16)
    gcols = sum(seg["nslots"] // 16 for seg in segments)
    scols = gcols
    gidx_all = np.zeros((n_cores, 128, gcols), dtype=np.int16)
    sidx_all = np.full((n_cores, 128, scols), dump_row, dtype=np.int16)

    seg_gcol0 = []
    seg_scol0 = []
    g0 = s0 = 0
    for seg in segments:
        seg_gcol0.append(g0)
        seg_scol0.append(s0)
        g0 += seg["nslots"] // 16
        s0 += seg["nslots"] // 16

    for cidx in range(n_cores):
        p = per_core[cidx]
        order = np.lexsort((p["row"], p["k"], p["chunk"], p["rnd"]))
        rnd_s, ch_s, k_s = p["rnd"][order], p["chunk"][order], p["k"][order]
        idx_s, row_s = p["idx"][order], p["row"][order]
        # slot of each pair: group_slot0 + position within group
        key = (rnd_s.astype(np.int64) * nchunk + ch_s) * K + k_s
        n = len(key)
        first = np.ones(n, dtype=bool)
        first[1:] = key[1:] != key[:-1]
        grp_start = np.maximum.accumulate(np.where(first, np.arange(n), 0))
        pos_in_grp = np.arange(n) - grp_start
        base = np.array(
            [group_slot0[(int(r_), int(c_), int(k_))]
             for r_, c_, k_ in zip(rnd_s[first], ch_s[first], k_s[first])],
            dtype=np.int64,
        )
        base_full = np.repeat(base, np.diff(np.nonzero(
            np.concatenate([first, [True]]))[0]))
        slots = base_full + pos_in_grp

        gvals = np.zeros(total_slots, dtype=np.int16)
        svals = np.full(total_slots, dump_row, dtype=np.int16)
        gvals[slots] = idx_s.astype(np.int16)
        svals[slots] = row_s
        # per-segment packing
        for si, seg in enumerate(segments):
            a, b = seg["slot0"], seg["slot0"] + seg["nslots"]
            gseg = gvals[a:b].reshape(-1, 16).T  # [16, n/16]
            gidx_all[cidx, :, seg_gcol0[si]:seg_gcol0[si] + (b - a) // 16] = (
                np.tile(gseg, (8, 1)))
            sseg = np.tile(svals[a:b].astype(np.int16).reshape(-1, 16).T,
                           (8, 1))  # wrapped like gather idxs
            sidx_all[cidx, :, seg_scol0[si]:seg_scol0[si] + (b - a) // 16] = sseg

    plan = dict(
        R=R, nchunk=nchunk, K=K, rows_per_core=rows_per_core,
        acc_rows=acc_rows, acc_total=acc_total, dump_row=dump_row,
        expand=expand,
        segments=segments, seg_gcol0=seg_gcol0, seg_scol0=seg_scol0,
        gcols=gcols, scols=scols, total_slots=total_slots,
    )
    return plan, gidx_all, sidx_all


def _build(plan, n_out, ftab_rows, n_cores):
    """Trace the Bass program. Returns nc."""
    nc = bacc.Bacc("TRN2", target_bir_lowering=False, debug=False)

    R, nchunk, K = plan["R"], plan["nchunk"], plan["K"]
    acc_rows, acc_total = plan["acc_rows"], plan["acc_total"]
    segments = plan["segments"]
    Cout = 64

    ftab = nc.dram_tensor("ftab", [ftab_rows, 128], BF16, kind="ExternalInput")
    wt = nc.dram_tensor("wt", [128, K * Cout], BF16, kind="ExternalInput")
    gidx = nc.dram_tensor("gidx", [128, plan["gcols"]], I16, kind="ExternalInput")
    sidx = nc.dram_tensor("sidx", [128, plan["scols"]], I16, kind="ExternalInput")
    gb = nc.dram_tensor("gb", [2, Cout], F32, kind="ExternalInput")
    # two accumulator banks cycled by round parity: scatter calls to
    # different banks have no WAW conflict, so adjacent rounds overlap
    acc0 = nc.dram_tensor("acc0", [acc_total, Cout], F32)
    acc1 = nc.dram_tensor("acc1", [acc_total, Cout], F32)
    accs = [acc0, acc1]
    cc_in = nc.dram_tensor("cc_in", [2, Cout], F32)
    cc_out = nc.dram_tensor("cc_out", [2, Cout], F32, addr_space="Shared")
    y = nc.dram_tensor("y", [acc_rows, Cout], F32, kind="ExternalOutput")

    Tb = acc_rows // 128  # BN column tiles

    with tile.TileContext(nc) as tc:
        with (
            tc.tile_pool(name="const", bufs=1) as cpool,
            tc.tile_pool(name="gpool", bufs=3) as gpool,
            tc.tile_pool(name="slab", bufs=3) as slabpool,
            tc.tile_pool(name="gixp", bufs=3) as gixpool,
            tc.tile_pool(name="sixp", bufs=3) as sixpool,
            tc.tile_pool(name="psum", bufs=8, space="PSUM") as pspool,
        ):
            # constants
            w_sb = cpool.tile([128, K * Cout], BF16, tag="w")
            nc.sync.dma_start(out=w_sb[:, :], in_=wt[:, :])
            zed = cpool.tile([128, 3200], F32, tag="zed")
            nc.vector.memset(zed[:, :], 0.0)
            # zero-init acc (acc_total*64 elems, in chunks of 128*3200)
            zrows = 128 * 3200 // Cout  # 6400 rows per DMA
            for bank in accs:
                r0 = 0
                while r0 < acc_total:
                    rcnt = min(zrows, acc_total - r0)
                    nc.sync.dma_start(
                        out=bank[r0:r0 + rcnt, :],
                        in_=zed[:, :rcnt * Cout // 128],
                    )
                    r0 += rcnt

            # main pipeline over segments
            for si, seg in enumerate(segments):
                ns = seg["nslots"]
                c = seg["c"]
                gi = gixpool.tile([128, SEG_SLOTS // 16], I16, tag="gi")
                nc.sync.dma_start(
                    out=gi[:, :ns // 16],
                    in_=gidx[:, plan["seg_gcol0"][si]:plan["seg_gcol0"][si] + ns // 16],
                )
                g = gpool.tile([128, 1, SEG_SLOTS], BF16, tag="g")
                nc.gpsimd.dma_gather(
                    out_ap=g[:, :, :ns],
                    in_ap=ftab[c * CHUNK:min((c + 1) * CHUNK, ftab_rows), :],
                    idxs_ap=gi[:, :ns // 16],
                    num_idxs=ns,
                    num_idxs_reg=ns,
                    elem_size=128,
                    transpose=True,
                )
                slab = slabpool.tile([128, SEG_SLOTS // 128, Cout], F32, tag="slab")
                for (k, cap, off) in seg["groups"]:
                    for j in range(cap // 128):
                        col = off + j * 128
                        ps = pspool.tile([128, Cout], F32, tag="ps")
                        nc.tensor.matmul(
                            out=ps[:, :],
                            lhsT=g[:, 0, col:col + 128],
                            rhs=w_sb[:, k * Cout:(k + 1) * Cout],
                            start=True, stop=True,
                        )
                        nc.vector.tensor_copy(
                            out=slab[:, col // 128, :], in_=ps[:, :])
                si_t = sixpool.tile([128, SEG_SLOTS // 16], I16, tag="si")
                nc.sync.dma_start(
                    out=si_t[:, :ns // 16],
                    in_=sidx[:, plan["seg_scol0"][si]:plan["seg_scol0"][si] + ns // 16],
                )
                nc.gpsimd.dma_scatter_add(
                    out_ap=accs[seg["r"] % 2][:, :],
                    in_ap=slab[:, :ns // 128, :],
                    idxs_ap=si_t[:, :ns // 16],
                    num_idxs=ns,
                    num_idxs_reg=ns,
                    elem_size=64,
                )

        # ---- BN phase ----
        with (
            tc.tile_pool(name="bn", bufs=1) as bnpool,
            tc.tile_pool(name="bns", bufs=4) as bnspool,
            tc.tile_pool(name="bnp", bufs=2, space="PSUM") as bnps,
        ):
            out_sb = bnpool.tile([128, Tb, 64], F32, tag="outsb")
            nc.sync.dma_start(out=out_sb[:, :, :], in_=acc0[0:acc_rows, :])
            bank_sb = bnpool.tile([128, Tb, 64], F32, tag="bank")
            nc.sync.dma_start(out=bank_sb[:, :, :], in_=acc1[0:acc_rows, :])
            nc.vector.tensor_tensor(
                out=out_sb[:, :, :], in0=out_sb[:, :, :],
                in1=bank_sb[:, :, :], op=mybir.AluOpType.add)
            ones = bnpool.tile([128, 1], F32, tag="ones")
            nc.vector.memset(ones[:, :], 1.0)
            sum_ps = bnps.tile([1, 64], F32, tag="sum")
            sq_ps = bnps.tile([1, 64], F32, tag="sq")
            for t in range(Tb):
                nc.tensor.matmul(
                    out=sum_ps[:, :], lhsT=ones[:, :], rhs=out_sb[:, t, :],
                    start=(t == 0), stop=(t == Tb - 1),
                )
            sqt = bnspool.tile([128, 64], F32, tag="sqt")
            for t in range(Tb):
                nc.vector.tensor_tensor(
                    out=sqt[:, :], in0=out_sb[:, t, :], in1=out_sb[:, t, :],
                    op=mybir.AluOpType.mult)
                nc.tensor.matmul(
                    out=sq_ps[:, :], lhsT=ones[:, :], rhs=sqt[:, :],
                    start=(t == 0), stop=(t == Tb - 1),
                )
            st0 = bnspool.tile([1, 64], F32, tag="st0")
            st1 = bnspool.tile([1, 64], F32, tag="st1")
            nc.vector.tensor_copy(out=st0[:, :], in_=sum_ps[:, :])
            nc.vector.tensor_copy(out=st1[:, :], in_=sq_ps[:, :])
            nc.sync.dma_start(out=cc_in[0:1, :], in_=st0[:, :])
            nc.sync.dma_start(out=cc_in[1:2, :], in_=st1[:, :])
            nc.gpsimd.collective_compute(
                "AllReduce",
                mybir.AluOpType.add,
                ins=[cc_in[:, :]],
                outs=[cc_out[:, :]],
                replica_groups=[list(range(n_cores))],
            )
            gs0 = bnspool.tile([1, 64], F32, tag="gs0")
            gs1 = bnspool.tile([1, 64], F32, tag="gs1")
            nc.sync.dma_start(out=gs0[:, :], in_=cc_out[0:1, :])
            nc.sync.dma_start(out=gs1[:, :], in_=cc_out[1:2, :])
            gam_t = bnspool.tile([1, 64], F32, tag="gam")
            bet_t = bnspool.tile([1, 64], F32, tag="bet")
            nc.sync.dma_start(out=gam_t[:, :], in_=gb[0:1, :])
            nc.sync.dma_start(out=bet_t[:, :], in_=gb[1:2, :])

            inv_n = 1.0 / float(n_out)
            mean_t = bnspool.tile([1, 64], F32, tag="mean")
            ex2_t = bnspool.tile([1, 64], F32, tag="ex2")
            var_t = bnspool.tile([1, 64], F32, tag="var")
            sd_t = bnspool.tile([1, 64], F32, tag="sd")
            rs_t = bnspool.tile([1, 64], F32, tag="rs")
            a_t = bnspool.tile([1, 64], F32, tag="a")
            b_t = bnspool.tile([1, 64], F32, tag="b")
            nc.vector.tensor_scalar_mul(mean_t[:, :], gs0[:, :], inv_n)
            nc.vector.tensor_scalar_mul(ex2_t[:, :], gs1[:, :], inv_n)
            nc.vector.tensor_tensor(
                out=var_t[:, :], in0=mean_t[:, :], in1=mean_t[:, :],
                op=mybir.AluOpType.mult)
            nc.vector.tensor_tensor(
                out=var_t[:, :], in0=ex2_t[:, :], in1=var_t[:, :],
                op=mybir.AluOpType.subtract)
            nc.vector.tensor_scalar_add(var_t[:, :], var_t[:, :], BN_EPS)
            nc.scalar.activation(
                out=sd_t[:, :], in_=var_t[:, :],
                func=mybir.ActivationFunctionType.Sqrt)
            nc.vector.reciprocal(out=rs_t[:, :], in_=sd_t[:, :])
            nc.vector.tensor_tensor(
                out=a_t[:, :], in0=gam_t[:, :], in1=rs_t[:, :],
                op=mybir.AluOpType.mult)
            nc.vector.tensor_tensor(
                out=b_t[:, :], in0=mean_t[:, :], in1=a_t[:, :],
                op=mybir.AluOpType.mult)
            nc.vector.tensor_tensor(
                out=b_t[:, :], in0=bet_t[:, :], in1=b_t[:, :],
                op=mybir.AluOpType.subtract)
            # broadcast [1,64] -> [128,64] via PE (ones[1,128]^T @ row)
            ones_row = bnspool.tile([1, 128], F32, tag="ones_row")
            nc.vector.memset(ones_row[:, :], 1.0)
            a_full = bnspool.tile([128, 64], F32, tag="afull")
            b_full = bnspool.tile([128, 64], F32, tag="bfull")
            ab_ps = bnps.tile([128, 64], F32, tag="abps")
            nc.tensor.matmul(
                out=ab_ps[:, :], lhsT=ones_row[:, :], rhs=a_t[:, :],
                start=True, stop=True)
            nc.vector.tensor_copy(out=a_full[:, :], in_=ab_ps[:, :])
            nc.tensor.matmul(
                out=ab_ps[:, :], lhsT=ones_row[:, :], rhs=b_t[:, :],
                start=True, stop=True)
            nc.vector.tensor_copy(out=b_full[:, :], in_=ab_ps[:, :])
            for t in range(Tb):
                nc.vector.tensor_tensor(
                    out=out_sb[:, t, :], in0=out_sb[:, t, :], in1=a_full[:, :],
                    op=mybir.AluOpType.mult)
                nc.vector.tensor_tensor(
                    out=out_sb[:, t, :], in0=out_sb[:, t, :], in1=b_full[:, :],
                    op=mybir.AluOpType.add)
                nc.scalar.activation(
                    out=out_sb[:, t, :], in_=out_sb[:, t, :],
                    func=mybir.ActivationFunctionType.Relu)
            nc.sync.dma_start(out=y[:, :], in_=out_sb[:, :, :])

    nc.compile()
    return nc


def _prepare(feats, W, gamma, beta, in_map, out_map, n_out, n_cores, dup_safe,
             expand=1):
    """Host prep shared by kernel() and tests. Returns (nc, in_maps, plan)."""
    n_out = int(n_out)
    K, Cin, Cout = W.shape
    assert Cin == 64 and Cout == 64
    in_map = np.asarray(in_map, dtype=np.int64)
    out_map = np.asarray(out_map, dtype=np.int64)
    feats = np.asarray(feats, dtype=np.float32)
    W = np.asarray(W, dtype=np.float32)

    plan, gidx_all, sidx_all = _route(
        in_map, out_map, n_out, n_cores, dup_safe, expand)

    ftab_rows = _roundup(feats.shape[0], CHUNK)
    ftab = np.zeros((ftab_rows, 128), dtype=ml_dtypes.bfloat16)
    ftab[:feats.shape[0], :64] = feats.astype(ml_dtypes.bfloat16)

    # W padded: [128 ic, K*64] bf16, rows 64..127 zero
    wt = np.zeros((128, K * 64), dtype=ml_dtypes.bfloat16)
    wt[:64, :] = (
        W.transpose(1, 0, 2).reshape(64, K * 64).astype(ml_dtypes.bfloat16))

    gb = np.stack([np.asarray(gamma, np.float32),
                   np.asarray(beta, np.float32)])

    nc = _build(plan, n_out, ftab_rows, n_cores)
    in_maps = [
        dict(ftab=ftab, wt=wt, gidx=gidx_all[c], sidx=sidx_all[c], gb=gb)
        for c in range(n_cores)
    ]
    return nc, in_maps, plan


def kernel(feats, W, gamma, beta, in_map, out_map, n_out):
    from concourse.bass_utils import run_bass_kernel_spmd

    n_cores = 8
    dup_safe = os.environ.get("DECONV_DUP_SAFE", "0") == "1"
    expand = int(os.environ.get("DECONV_EXPAND", "1"))
    nc, in_maps, plan = _prepare(
        feats, W, gamma, beta, in_map, out_map, n_out, n_cores, dup_safe,
        expand)
    res = run_bass_kernel_spmd(nc, in_maps, list(range(n_cores)))
    rows = plan["rows_per_core"]
    out = np.concatenate(
        [res.results[c]["y"][:rows] for c in range(n_cores)], axis=0)
    return out.astype(np.float32)



# revision 11
# speedup vs baseline: 2.1173x; 2.1173x over previous
"""Trainium2 Bass kernel for nn_BasicDeconvolutionBlock (two-phase design).

Reference computation:
    gathered = feats[in_map]                         # [K, M, Cin]
    contrib  = einsum('kmc,kcd->kmd', gathered, W)   # [K, M, Cout]
    out      = zeros([n_out, Cout]).at[out_map].add(contrib)
    y        = relu(batchnorm(out))                  # batch stats over n_out rows

Strategy (8 NeuronCores, SPMD, output-row sharding):
  Host routes each (k, m) pair to the core owning its output row
  (row blocks of n_out/8, ~169k pairs/core), orders the pairs by
  (feats-chunk, k, out-tile) with per-(chunk,k,out-tile) "cells" padded to
  even length, groups (chunk,k) padded to 128.

  Phase A (gather-GEMM): SWDGE dma_gather (transpose) pulls feats rows
  (fp16, 256B) channel-major; per-128-slot matmul against W[k] (fp16);
  PSUM -> fp16 slab (scalar engine Copy) -> contiguous HBM contrib table
  (slot-major 128B rows, in window tensors of 64k slots so phase B's
  int16 gather indices stay in range).  No scatter-add, no occurrence
  rounds.

  Phase B (gather-reduce): contrib rows are fetched in out-tile order as
  PAIRS (256B descriptors = 2 rows, halving descriptor count; cells are
  even-aligned so pairs never straddle cells).  One-hot S matrices
  ([128 pairs x 128 rows], fp16) are built on-chip with a single
  broadcast is_equal against an iota tile per gather call; matmul
  lhsT=S, rhs=gathered pair block accumulates the segmented scatter-add
  directly in PSUM per 128-row out-tile.  Tiles accumulate across the
  window passes into an SBUF fp32 slab.

  BN: per-tile ones-matmul row sums + sum of squares, [2,64] AllReduce
  across 8 cores, batched normalize + ReLU, output shard [25088,64] fp32.
"""

import numpy as np

BN_EPS = 1e-5
SEG = 896            # max descriptors per SWDGE call (Q7 ucode ring limit)
CHUNK = 32768        # int16 gather index range per feats chunk
WSLOTS = 65536       # contrib-table window: 32768 pairs of slots


def _lazy():
    global F32, F16, I16, mybir, bacc, tile
    import sys
    for p in ("/opt/trn_rl_repo",):
        if p not in sys.path:
            sys.path.insert(0, p)
    from concourse import bacc as _bacc, mybir as _mybir
    import concourse.tile as _tile
    mybir, bacc, tile = _mybir, _bacc, _tile
    F32 = mybir.dt.float32
    F16 = mybir.dt.float16
    I16 = mybir.dt.int16


def _roundup(x, m):
    return (x + m - 1) // m * m


def _plan(in_map, out_map, n_out, n_cores, rows_per_core, chunk, wslots):
    """Host-side routing. Returns a dict plan + per-core packed arrays."""
    K, M = in_map.shape
    tiles = _roundup(rows_per_core, 128) // 128
    wpairs = wslots // 2
    in_flat = np.asarray(in_map).ravel().astype(np.int64)
    out_flat = np.asarray(out_map).ravel().astype(np.int64)
    k_idx = np.repeat(np.arange(K, dtype=np.int64), M)
    core = out_flat // rows_per_core
    row_local = out_flat - core * rows_per_core
    t_idx = row_local >> 7
    r128 = row_local & 127
    chnk = in_flat // chunk
    idx_local = in_flat - chnk * chunk
    nchunk = int(chnk.max()) + 1
    NG = nchunk * K
    NCELLS = NG * tiles
    cell = (chnk * K + k_idx) * tiles + t_idx

    sizes = np.zeros((n_cores, NCELLS), np.int64)
    for c in range(n_cores):
        sizes[c] = np.bincount(cell[core == c], minlength=NCELLS)
    ce = (sizes + 1) // 2 * 2                       # cell sizes even-padded
    gsz = ce.reshape(n_cores, NG, tiles).sum(-1)
    gcap = _roundup(gsz.max(0), 128)                # [NG] shared
    gstart = np.zeros(NG + 1, np.int64)
    gstart[1:] = np.cumsum(gcap)
    AS = int(gstart[-1])                            # total A slots
    nwin = _roundup(AS, wslots) // wslots

    ce3 = ce.reshape(n_cores, NG, tiles)
    cstart = (np.cumsum(ce3, axis=2) - ce3
              + gstart[None, :NG, None])            # [cores, NG, tiles]
    cstart_f = cstart.reshape(n_cores, NCELLS)

    order = np.lexsort((cell, core))
    cell_s, core_s = cell[order], core[order]
    key = core_s * NCELLS + cell_s
    n = len(key)
    first = np.ones(n, bool)
    first[1:] = key[1:] != key[:-1]
    gs = np.maximum.accumulate(np.where(first, np.arange(n), 0))
    pos = np.arange(n) - gs
    Apos_s = cstart_f[core_s, cell_s] + pos

    gidxA = np.zeros((n_cores, AS), np.int16)
    rowsA = np.full((n_cores, AS), 255, np.int16)
    gidxA[core_s, Apos_s] = idx_local[order].astype(np.int16)
    rowsA[core_s, Apos_s] = r128[order].astype(np.int16)

    # ---- B stream ----
    p0 = cstart_f // 2
    cnt2 = ce.reshape(n_cores, NCELLS) // 2          # pairs per cell
    t_of_cell = np.tile(np.arange(tiles, dtype=np.int64), NG)

    sizes_B = np.zeros((n_cores, nwin, tiles), np.int64)
    for c in range(n_cores):
        for w in range(nwin):
            lo, hi = w * wpairs, (w + 1) * wpairs
            ov = np.clip(np.minimum(p0[c] + cnt2[c], hi)
                         - np.maximum(p0[c], lo), 0, None)
            sizes_B[c, w] = ov.reshape(NG, tiles).sum(0)
    CB = sizes_B.max(0)                              # [nwin, tiles] shared
    sec = CB.sum(1)
    secpad = _roundup(sec, 128)
    wstart = np.zeros(nwin + 1, np.int64)
    wstart[1:] = np.cumsum(secpad)
    BS = int(wstart[-1])                             # total B pairs
    P_wt = np.cumsum(CB, axis=1) - CB + wstart[:nwin, None]

    bidx = np.zeros((n_cores, BS), np.int16)
    brow = np.full((n_cores, BS, 2), 255, np.int16)
    for c in range(n_cores):
        tot = int(cnt2[c].sum())
        if tot == 0:
            continue
        cums = np.cumsum(cnt2[c]) - cnt2[c]
        ap_all = (np.repeat(p0[c], cnt2[c])
                  + np.arange(tot) - np.repeat(cums, cnt2[c]))
        t_all = np.repeat(t_of_cell, cnt2[c])
        w_all = ap_all // wpairs
        key2 = w_all * tiles + t_all
        o2 = np.lexsort((ap_all, key2))
        k2, a2, t2, w2 = key2[o2], ap_all[o2], t_all[o2], w_all[o2]
        f2 = np.ones(tot, bool)
        f2[1:] = k2[1:] != k2[:-1]
        gs2 = np.maximum.accumulate(np.where(f2, np.arange(tot), 0))
        pos2 = np.arange(tot) - gs2
        bpos = P_wt[w2, t2] + pos2
        bidx[c, bpos] = (a2 - w2 * wpairs).astype(np.int16)
        brow[c, bpos, 0] = rowsA[c, a2 * 2]
        brow[c, bpos, 1] = rowsA[c, a2 * 2 + 1]

    # entries (block, w, t) + per-(w,t) entry spans
    entries = []
    ent_span = {}
    for w in range(nwin):
        for t in range(tiles):
            if CB[w, t] == 0:
                continue
            b0 = int(P_wt[w, t]) // 128
            b1 = (int(P_wt[w, t]) + int(CB[w, t]) + 127) // 128
            ent_span[(w, t)] = (len(entries), len(entries) + b1 - b0)
            entries.extend((b, w, t) for b in range(b0, b1))
    NENT = len(entries)

    metaI = np.full((n_cores, NENT, 128, 2), 255, np.int16)
    for e, (b, w, t) in enumerate(entries):
        lo = max(b * 128, int(P_wt[w, t]))
        hi = min((b + 1) * 128, int(P_wt[w, t]) + int(CB[w, t]))
        if hi > lo:
            metaI[:, e, lo - b * 128:hi - b * 128, :] = brow[:, lo:hi, :]
    meta = metaI.transpose(0, 2, 1, 3).astype(np.float16)

    # ---- segmentation ----
    # A calls: cut at chunk-section and window boundaries, then SEG slots.
    csec = [int(gstart[ci * K]) for ci in range(nchunk + 1)]
    bounds = sorted(set(
        csec + [w * wslots for w in range(nwin + 1) if w * wslots <= AS]
        + [AS]))
    k_of_group = np.tile(np.arange(K, dtype=np.int64), nchunk)
    k_of_slot = np.repeat(k_of_group, gcap)
    a_calls = []   # (slot0, ns, chunk_id, win, [k per tile])
    for lo, hi in zip(bounds[:-1], bounds[1:]):
        s = lo
        while s < hi:
            ns = min(SEG, hi - s)
            ks = [int(k_of_slot[s + 128 * j]) for j in range(ns // 128)]
            a_calls.append(
                (s, ns, int(np.searchsorted(csec, s, side="right") - 1),
                 s // wslots, ks))
            s += ns

    # B calls: per window section, SEG-pair chunks (128-multiples)
    b_calls = []   # (pair0, np_, w)
    for w in range(nwin):
        s = int(wstart[w])
        hi = int(wstart[w + 1])
        while s < hi:
            np_ = min(SEG, hi - s)
            b_calls.append((s, np_, w))
            s += np_
    call_lo = np.array([c[0] for c in b_calls])
    call_ent = [[] for _ in b_calls]
    for e, (b, w, t) in enumerate(entries):
        ci = int(np.searchsorted(call_lo, b * 128, side="right") - 1)
        assert b_calls[ci][0] <= b * 128 < b_calls[ci][0] + b_calls[ci][1]
        call_ent[ci].append(e)
    ne_max = max((len(x) for x in call_ent), default=0)

    first_w = {}
    for t in range(tiles):
        for w in range(nwin):
            if CB[w, t] > 0:
                first_w[t] = w
                break

    gidxA_w = np.zeros((n_cores, 128, AS // 16), np.int16)
    gidxB_w = np.zeros((n_cores, 128, BS // 16), np.int16)
    for c in range(n_cores):
        gidxA_w[c] = np.tile(gidxA[c].reshape(-1, 16).T, (8, 1))
        gidxB_w[c] = np.tile(bidx[c].reshape(-1, 16).T, (8, 1))

    plan = dict(
        K=K, tiles=tiles, nchunk=nchunk, nwin=nwin, AS=AS, BS=BS,
        chunk=chunk, wslots=wslots, rows_per_core=rows_per_core,
        a_calls=a_calls, b_calls=b_calls, entries=entries,
        ent_span=ent_span, call_ent=call_ent, ne_max=ne_max,
        first_w=first_w, NENT=NENT, n_out=int(n_out),
    )
    arrays = dict(gidxA=gidxA_w, gidxB=gidxB_w, meta=meta)
    return plan, arrays


def _build(plan, n_cores, ftab_rows):
    """Trace the Bass program."""
    _lazy()
    nc = bacc.Bacc("TRN2", target_bir_lowering=False, debug=False)

    K, tiles, nwin = plan["K"], plan["tiles"], plan["nwin"]
    AS, BS, NENT = plan["AS"], plan["BS"], plan["NENT"]
    ne_max = max(plan["ne_max"], 1)
    wslots = plan["wslots"]
    chunk = plan["chunk"]
    n_out = plan["n_out"]
    Cout = 64
    rows_pad = tiles * 128

    ftab = nc.dram_tensor("ftab", [ftab_rows, 128], F16, kind="ExternalInput")
    wt = nc.dram_tensor("wt", [128, K * Cout], F16, kind="ExternalInput")
    gidxA = nc.dram_tensor("gidxA", [128, AS // 16], I16, kind="ExternalInput")
    gidxB = nc.dram_tensor("gidxB", [128, BS // 16], I16, kind="ExternalInput")
    meta = nc.dram_tensor("meta", [128, NENT, 2], F16, kind="ExternalInput")
    iota2 = nc.dram_tensor("iota2", [128, 128, 2], F16,
                           kind="ExternalInput")
    gb = nc.dram_tensor("gb", [2, Cout], F32, kind="ExternalInput")
    atabs = [nc.dram_tensor(f"atab{w}", [wslots // 2, 128], F16)
             for w in range(nwin)]
    cc_in = nc.dram_tensor("cc_in", [2, Cout], F32)
    cc_out = nc.dram_tensor("cc_out", [2, Cout], F32, addr_space="Shared")
    y = nc.dram_tensor("y", [rows_pad, Cout], F32, kind="ExternalOutput")

    # slot-major write views of the contrib windows
    atv = [a[:, :].flatten().rearrange("(t p c) -> p t c", p=128, c=64)
           for a in atabs]

    entries = plan["entries"]
    ent_span = plan["ent_span"]
    call_ent = plan["call_ent"]
    first_w = plan["first_w"]
    ent_first = {}
    ent_last = {}
    for (w, t), (e0, e1) in ent_span.items():
        ent_first[(w, t)] = e0
        ent_last[(w, t)] = e1 - 1

    with tile.TileContext(nc) as tc:
        with (
            tc.tile_pool(name="const", bufs=1) as cpool,
            tc.tile_pool(name="agix", bufs=3) as agix,
            tc.tile_pool(name="ag", bufs=3) as agp,
            tc.tile_pool(name="aslab", bufs=3) as aslab,
            tc.tile_pool(name="bgix", bufs=3) as bgix,
            tc.tile_pool(name="bg", bufs=3) as bgp,
            tc.tile_pool(name="bmeta", bufs=3) as bmeta,
            tc.tile_pool(name="bs", bufs=3) as bspool,
            tc.tile_pool(name="slab", bufs=1) as slabpool,
        ):
            w_sb = cpool.tile([128, K * Cout], F16, tag="w")
            nc.sync.dma_start(out=w_sb[:, :], in_=wt[:, :])
            iota_sb = cpool.tile([128, 128, 2], F16, tag="iota")
            nc.sync.dma_start(out=iota_sb[:, :, :], in_=iota2[:, :, :])
            out_slab = slabpool.tile([128, tiles, Cout], F32, tag="slab")

            a_by_w = [[] for _ in range(nwin)]
            for call in plan["a_calls"]:
                a_by_w[call[3]].append(call)
            b_by_w = [[] for _ in range(nwin)]
            for ci, call in enumerate(plan["b_calls"]):
                b_by_w[call[2]].append((ci, call))

            psum_of = {}
            apsum, bpsum = [], []

            ABATCH = 8

            def a_batches(w):
                calls = a_by_w[w]
                outs = []
                for i0 in range(0, len(calls), ABATCH):
                    outs.append(_mk_a(calls[i0:i0 + ABATCH], w, i0))
                return outs

            def _mk_a(batch, w, i0):
                def go():
                    bs0 = batch[0][0]
                    bs1 = batch[-1][0] + batch[-1][1]
                    gib = agix.tile([128, ABATCH * SEG // 16], I16, tag="agi")
                    ldq = nc.scalar if (i0 // ABATCH) % 2 else nc.sync
                    ldq.dma_start(
                        out=gib[:, :(bs1 - bs0) // 16],
                        in_=gidxA[:, bs0 // 16:bs1 // 16])
                    for (s0, ns, ch, _w, ks) in batch:
                        nt = ns // 128
                        g = agp.tile([128, 1, SEG], F16, tag="ag")
                        nc.gpsimd.dma_gather(
                            out_ap=g[:, :, :ns],
                            in_ap=ftab[ch * chunk:(ch + 1) * chunk, :],
                            idxs_ap=gib[:, (s0 - bs0) // 16:
                                        (s0 - bs0 + ns) // 16],
                            num_idxs=ns,
                            num_idxs_reg=ns,
                            elem_size=128,
                            transpose=True,
                        )
                        ps = apsum[0].tile(
                            [128, SEG // 128, Cout], F32, tag="aps")
                        for j in range(nt):
                            nc.tensor.matmul(
                                out=ps[:, j, :],
                                lhsT=g[:, 0, j * 128:(j + 1) * 128],
                                rhs=w_sb[:, ks[j] * Cout:
                                         (ks[j] + 1) * Cout],
                                start=True, stop=True,
                            )
                        sl = aslab.tile(
                            [128, SEG // 128, Cout], F16, tag="asl")
                        nc.scalar.activation(
                            out=sl[:, :nt, :], in_=ps[:, :nt, :],
                            func=mybir.ActivationFunctionType.Copy)
                        t0 = (s0 - w * wslots) // 128
                        wq = nc.sync if (s0 // SEG) % 2 else nc.scalar
                        wq.dma_start(
                            out=atv[w][:, t0:t0 + nt, :], in_=sl[:, :nt, :])
                return go

            BBATCH = 8

            def b_batches(w):
                calls = b_by_w[w]
                outs = []
                for i0 in range(0, len(calls), BBATCH):
                    outs.append(_mk_b(calls[i0:i0 + BBATCH], w, i0))
                return outs

            def _mk_b(bat, w, i0):
                def go():
                    bp0 = bat[0][1][0]
                    bp1 = bat[-1][1][0] + bat[-1][1][1]
                    gib = bgix.tile([128, BBATCH * SEG // 16], I16, tag="bgi")
                    ldq = nc.scalar if (i0 // BBATCH) % 2 else nc.sync
                    ldq.dma_start(
                        out=gib[:, :(bp1 - bp0) // 16],
                        in_=gidxB[:, bp0 // 16:bp1 // 16])
                    be_lo = call_ent[bat[0][0]][0]
                    be_hi = call_ent[bat[-1][0]][-1] + 1
                    mtb = bmeta.tile([128, BBATCH * ne_max, 2], F16, tag="bm")
                    nc.scalar.dma_start(
                        out=mtb[:, :be_hi - be_lo, :],
                        in_=meta[:, be_lo:be_hi, :])
                    for (ci, (pair0, np_, _w)) in bat:
                        nb = np_ // 128
                        g = bgp.tile([128, SEG // 128, 128], F16, tag="bg")
                        nc.gpsimd.dma_gather(
                            out_ap=g[:, :nb, :],
                            in_ap=atabs[w][:, :],
                            idxs_ap=gib[:, (pair0 - bp0) // 16:
                                        (pair0 - bp0 + np_) // 16],
                            num_idxs=np_,
                            num_idxs_reg=np_,
                            elem_size=128,
                        )
                        ents = call_ent[ci]
                        if not ents:
                            continue
                        ne = len(ents)
                        e_lo = ents[0]
                        st = bspool.tile(
                            [128, 128, ne_max, 2], F16, tag="bsl")
                        nc.vector.tensor_tensor(
                            out=st[:, :, :ne, :],
                            in0=mtb[:, e_lo - be_lo:e_lo - be_lo + ne, :]
                            .unsqueeze(1).broadcast_to([128, 128, ne, 2]),
                            in1=iota_sb[:, :, :].unsqueeze(2)
                            .broadcast_to([128, 128, ne, 2]),
                            op=mybir.AluOpType.is_equal,
                        )
                        for e in ents:
                            b, we, t = entries[e]
                            if e == ent_first[(we, t)]:
                                psum_of[(we, t)] = bpsum[0].tile(
                                    [128, Cout], F32, tag="bps",
                                    name=f"bps_{we}_{t}")
                            ps = psum_of[(we, t)]
                            last = e == ent_last[(we, t)]
                            for eo in range(2):
                                nc.tensor.matmul(
                                    out=ps[:, :],
                                    lhsT=st[:, :, e - e_lo, eo],
                                    rhs=g[:, b - pair0 // 128,
                                          eo * 64:(eo + 1) * 64],
                                    start=(e == ent_first[(we, t)]
                                           and eo == 0),
                                    stop=(last and eo == 1),
                                )
                            if last:
                                if first_w[t] == we:
                                    nc.vector.tensor_copy(
                                        out=out_slab[:, t, :], in_=ps[:, :])
                                else:
                                    nc.vector.tensor_tensor(
                                        out=out_slab[:, t, :],
                                        in0=out_slab[:, t, :], in1=ps[:, :],
                                        op=mybir.AluOpType.add)
                                del psum_of[(we, t)]
                return go

            with (
                tc.tile_pool(name="apsum", bufs=3, space="PSUM") as apsum_,
                tc.tile_pool(name="bpsum", bufs=4, space="PSUM") as bpsum_,
            ):
                apsum.append(apsum_)
                bpsum.append(bpsum_)
                for go in a_batches(0):
                    go()
                for w in range(1, nwin):
                    A, B = a_batches(w), b_batches(w - 1)
                    na, nb = len(A), len(B)
                    ia = ib = 0
                    while ia < na or ib < nb:
                        if ia < na and (ib >= nb or ia * nb <= ib * na):
                            A[ia]()
                            ia += 1
                        else:
                            B[ib]()
                            ib += 1
                for go in b_batches(nwin - 1):
                    go()

            # ---- BN + ReLU ----
            with (
                tc.tile_pool(name="bn", bufs=4) as bnp,
                tc.tile_pool(name="bnps", bufs=2, space="PSUM") as bnps,
            ):
                ones = bnp.tile([128, 1], F32, tag="ones")
                nc.vector.memset(ones[:, :], 1.0)
                sum_ps = bnps.tile([1, Cout], F32, tag="sum")
                sq_ps = bnps.tile([1, Cout], F32, tag="sq")
                for t in range(tiles):
                    nc.tensor.matmul(
                        out=sum_ps[:, :], lhsT=ones[:, :],
                        rhs=out_slab[:, t, :],
                        start=(t == 0), stop=(t == tiles - 1))
                sqt = bnp.tile([128, Cout], F32, tag="sqt")
                for t in range(tiles):
                    nc.vector.tensor_tensor(
                        out=sqt[:, :], in0=out_slab[:, t, :],
                        in1=out_slab[:, t, :], op=mybir.AluOpType.mult)
                    nc.tensor.matmul(
                        out=sq_ps[:, :], lhsT=ones[:, :], rhs=sqt[:, :],
                        start=(t == 0), stop=(t == tiles - 1))
                st0 = bnp.tile([1, Cout], F32, tag="st0")
                st1 = bnp.tile([1, Cout], F32, tag="st1")
                nc.vector.tensor_copy(out=st0[:, :], in_=sum_ps[:, :])
                nc.vector.tensor_copy(out=st1[:, :], in_=sq_ps[:, :])
                nc.sync.dma_start(out=cc_in[0:1, :], in_=st0[:, :])
                nc.sync.dma_start(out=cc_in[1:2, :], in_=st1[:, :])
                nc.gpsimd.collective_compute(
                    "AllReduce",
                    mybir.AluOpType.add,
                    ins=[cc_in[:, :]],
                    outs=[cc_out[:, :]],
                    replica_groups=[list(range(n_cores))],
                )
                gs0 = bnp.tile([1, Cout], F32, tag="gs0")
                gs1 = bnp.tile([1, Cout], F32, tag="gs1")
                nc.sync.dma_start(out=gs0[:, :], in_=cc_out[0:1, :])
                nc.sync.dma_start(out=gs1[:, :], in_=cc_out[1:2, :])
                gam_t = bnp.tile([1, Cout], F32, tag="gam")
                bet_t = bnp.tile([1, Cout], F32, tag="bet")
                nc.sync.dma_start(out=gam_t[:, :], in_=gb[0:1, :])
                nc.sync.dma_start(out=bet_t[:, :], in_=gb[1:2, :])

                inv_n = 1.0 / float(n_out)
                mean_t = bnp.tile([1, Cout], F32, tag="mean")
                ex2_t = bnp.tile([1, Cout], F32, tag="ex2")
                var_t = bnp.tile([1, Cout], F32, tag="var")
                sd_t = bnp.tile([1, Cout], F32, tag="sd")
                rs_t = bnp.tile([1, Cout], F32, tag="rs")
                a_t = bnp.tile([1, Cout], F32, tag="a")
                b_t = bnp.tile([1, Cout], F32, tag="b")
                nc.vector.tensor_scalar_mul(mean_t[:, :], gs0[:, :], inv_n)
                nc.vector.tensor_scalar_mul(ex2_t[:, :], gs1[:, :], inv_n)
                nc.vector.tensor_tensor(
                    out=var_t[:, :], in0=mean_t[:, :], in1=mean_t[:, :],
                    op=mybir.AluOpType.mult)
                nc.vector.tensor_tensor(
                    out=var_t[:, :], in0=ex2_t[:, :], in1=var_t[:, :],
                    op=mybir.AluOpType.subtract)
                nc.vector.tensor_scalar_add(var_t[:, :], var_t[:, :], BN_EPS)
                nc.scalar.activation(
                    out=sd_t[:, :], in_=var_t[:, :],
                    func=mybir.ActivationFunctionType.Sqrt)
                nc.vector.reciprocal(out=rs_t[:, :], in_=sd_t[:, :])
                nc.vector.tensor_tensor(
                    out=a_t[:, :], in0=gam_t[:, :], in1=rs_t[:, :],
                    op=mybir.AluOpType.mult)
                nc.vector.tensor_tensor(
                    out=b_t[:, :], in0=mean_t[:, :], in1=a_t[:, :],
                    op=mybir.AluOpType.mult)
                nc.vector.tensor_tensor(
                    out=b_t[:, :], in0=bet_t[:, :], in1=b_t[:, :],
                    op=mybir.AluOpType.subtract)
                ones_row = bnp.tile([1, 128], F32, tag="ones_row")
                nc.vector.memset(ones_row[:, :], 1.0)
                a_full = bnp.tile([128, Cout], F32, tag="afull")
                b_full = bnp.tile([128, Cout], F32, tag="bfull")
                ab_ps = bnps.tile([128, Cout], F32, tag="abps")
                nc.tensor.matmul(
                    out=ab_ps[:, :], lhsT=ones_row[:, :], rhs=a_t[:, :],
                    start=True, stop=True)
                nc.vector.tensor_copy(out=a_full[:, :], in_=ab_ps[:, :])
                nc.tensor.matmul(
                    out=ab_ps[:, :], lhsT=ones_row[:, :], rhs=b_t[:, :],
                    start=True, stop=True)
                nc.vector.tensor_copy(out=b_full[:, :], in_=ab_ps[:, :])
                # batched normalize + relu over the whole slab
                nc.vector.tensor_tensor(
                    out=out_slab[:, :, :], in0=out_slab[:, :, :],
                    in1=a_full[:, :].unsqueeze(1)
                    .broadcast_to([128, tiles, Cout]),
                    op=mybir.AluOpType.mult)
                nc.vector.tensor_tensor(
                    out=out_slab[:, :, :], in0=out_slab[:, :, :],
                    in1=b_full[:, :].unsqueeze(1)
                    .broadcast_to([128, tiles, Cout]),
                    op=mybir.AluOpType.add)
                nc.scalar.activation(
                    out=out_slab[:, :, :], in_=out_slab[:, :, :],
                    func=mybir.ActivationFunctionType.Relu)
                yv = y[:, :].flatten().rearrange(
                    "(t p c) -> p t c", p=128, c=64)
                nc.sync.dma_start(out=yv, in_=out_slab[:, :, :])

    nc.compile()
    return nc


def _prepare(feats, W, gamma, beta, in_map, out_map, n_out,
             n_cores=8, dup_safe=False, expand=1):
    """Host prep shared by kernel() and tests. Returns (nc, in_maps, plan)."""
    _lazy()
    n_out = int(n_out)
    K, Cin, Cout = W.shape
    assert Cin == 64 and Cout == 64
    rows_per_core = n_out // n_cores
    assert rows_per_core * n_cores == n_out

    in_map = np.asarray(in_map, dtype=np.int64)
    out_map = np.asarray(out_map, dtype=np.int64)
    feats = np.asarray(feats, dtype=np.float32)
    W = np.asarray(W, dtype=np.float32)

    plan, arrays = _plan(in_map, out_map, n_out, n_cores, rows_per_core,
                         CHUNK, WSLOTS)

    ftab_rows = _roundup(feats.shape[0], CHUNK)
    ftab = np.zeros((ftab_rows, 128), dtype=np.float16)
    ftab[:feats.shape[0], :64] = feats.astype(np.float16)
    wt = np.zeros((128, K * 64), dtype=np.float16)
    wt[:64, :] = W.transpose(1, 0, 2).reshape(64, K * 64).astype(np.float16)
    gb = np.stack([np.asarray(gamma, np.float32),
                   np.asarray(beta, np.float32)])
    iota2 = np.broadcast_to(np.arange(128, dtype=np.float32)[None, :, None],
                            (128, 128, 2)).astype(np.float16)

    nc = _build(plan, n_cores, ftab_rows)
    in_maps = [
        dict(ftab=ftab, wt=wt, gb=gb, iota2=iota2,
             gidxA=arrays["gidxA"][c], gidxB=arrays["gidxB"][c],
             meta=arrays["meta"][c])
        for c in range(n_cores)
    ]
    return nc, in_maps, plan


def kernel(feats, W, gamma, beta, in_map, out_map, n_out):
    _lazy()
    from concourse.bass_utils import run_bass_kernel_spmd

    n_cores = 8
    nc, in_maps, plan = _prepare(
        feats, W, gamma, beta, in_map, out_map, n_out, n_cores)
    res = run_bass_kernel_spmd(nc, in_maps, list(range(n_cores)))
    rows = plan["rows_per_core"]
    out = np.concatenate(
        [res.results[c]["y"][:rows] for c in range(n_cores)], axis=0)
    return out.astype(np.float32)


# revision 14
# speedup vs baseline: 2.3099x; 1.0910x over previous
"""Trainium2 Bass kernel for nn_BasicDeconvolutionBlock (two-phase design).

Reference computation:
    gathered = feats[in_map]                         # [K, M, Cin]
    contrib  = einsum('kmc,kcd->kmd', gathered, W)   # [K, M, Cout]
    out      = zeros([n_out, Cout]).at[out_map].add(contrib)
    y        = relu(batchnorm(out))                  # batch stats over n_out rows

Strategy (8 NeuronCores, SPMD, output-row sharding):
  Host routes each (k, m) pair to the core owning its output row
  (row blocks of n_out/8, ~169k pairs/core), orders the pairs by
  (feats-chunk, k, out-tile) with per-(chunk,k,out-tile) "cells" padded to
  even length, groups (chunk,k) padded to 128.

  Phase A (gather-GEMM): SWDGE dma_gather (transpose) pulls feats rows
  (fp16, 256B) channel-major; per-128-slot matmul against W[k] (fp16);
  PSUM -> fp16 slab (scalar engine Copy) -> contiguous HBM contrib table
  (slot-major 128B rows, in window tensors of 64k slots so phase B's
  int16 gather indices stay in range).  No scatter-add, no occurrence
  rounds.

  Phase B (gather-reduce): contrib rows are fetched in out-tile order as
  PAIRS (256B descriptors = 2 rows, halving descriptor count; cells are
  even-aligned so pairs never straddle cells).  One-hot S matrices
  ([128 pairs x 128 rows], fp16) are built on-chip with a single
  broadcast is_equal against an iota tile per gather call; matmul
  lhsT=S, rhs=gathered pair block accumulates the segmented scatter-add
  directly in PSUM per 128-row out-tile.  Tiles accumulate across the
  window passes into an SBUF fp32 slab.

  BN: per-tile ones-matmul row sums + sum of squares, [2,64] AllReduce
  across 8 cores, batched normalize + ReLU, output shard [25088,64] fp32.
"""

import numpy as np

BN_EPS = 1e-5
SEG = 896            # max descriptors per SWDGE call (Q7 ucode ring limit)
CHUNK = 32768        # int16 gather index range per feats chunk
WSLOTS = 65536       # contrib-table window: 32768 pairs of slots


def _lazy():
    global F32, F16, I16, mybir, bacc, tile
    import sys
    for p in ("/opt/trn_rl_repo",):
        if p not in sys.path:
            sys.path.insert(0, p)
    from concourse import bacc as _bacc, mybir as _mybir
    import concourse.tile as _tile
    mybir, bacc, tile = _mybir, _bacc, _tile
    F32 = mybir.dt.float32
    F16 = mybir.dt.float16
    I16 = mybir.dt.int16


def _roundup(x, m):
    return (x + m - 1) // m * m


def _plan(in_map, out_map, n_out, n_cores, rows_per_core, chunk, wslots):
    """Host-side routing. Returns a dict plan + per-core packed arrays."""
    K, M = in_map.shape
    tiles = _roundup(rows_per_core, 128) // 128
    wpairs = wslots // 2
    in_flat = np.asarray(in_map).ravel().astype(np.int64)
    out_flat = np.asarray(out_map).ravel().astype(np.int64)
    k_idx = np.repeat(np.arange(K, dtype=np.int64), M)
    core = out_flat // rows_per_core
    row_local = out_flat - core * rows_per_core
    t_idx = row_local >> 7
    r128 = row_local & 127
    chnk = in_flat // chunk
    idx_local = in_flat - chnk * chunk
    nchunk = int(chnk.max()) + 1
    NG = nchunk * K
    NCELLS = NG * tiles
    cell = (chnk * K + k_idx) * tiles + t_idx

    sizes = np.zeros((n_cores, NCELLS), np.int64)
    for c in range(n_cores):
        sizes[c] = np.bincount(cell[core == c], minlength=NCELLS)
    ce = (sizes + 1) // 2 * 2                       # cell sizes even-padded
    gsz = ce.reshape(n_cores, NG, tiles).sum(-1)
    gcap = _roundup(gsz.max(0), 128)                # [NG] shared
    gstart = np.zeros(NG + 1, np.int64)
    gstart[1:] = np.cumsum(gcap)
    AS = int(gstart[-1])                            # total A slots
    nwin = _roundup(AS, wslots) // wslots

    ce3 = ce.reshape(n_cores, NG, tiles)
    cstart = (np.cumsum(ce3, axis=2) - ce3
              + gstart[None, :NG, None])            # [cores, NG, tiles]
    cstart_f = cstart.reshape(n_cores, NCELLS)

    order = np.lexsort((cell, core))
    cell_s, core_s = cell[order], core[order]
    key = core_s * NCELLS + cell_s
    n = len(key)
    first = np.ones(n, bool)
    first[1:] = key[1:] != key[:-1]
    gs = np.maximum.accumulate(np.where(first, np.arange(n), 0))
    pos = np.arange(n) - gs
    Apos_s = cstart_f[core_s, cell_s] + pos

    gidxA = np.zeros((n_cores, AS), np.int16)
    rowsA = np.full((n_cores, AS), 255, np.int16)
    gidxA[core_s, Apos_s] = idx_local[order].astype(np.int16)
    rowsA[core_s, Apos_s] = r128[order].astype(np.int16)

    # ---- B stream ----
    p0 = cstart_f // 2
    cnt2 = ce.reshape(n_cores, NCELLS) // 2          # pairs per cell
    t_of_cell = np.tile(np.arange(tiles, dtype=np.int64), NG)

    sizes_B = np.zeros((n_cores, nwin, tiles), np.int64)
    for c in range(n_cores):
        for w in range(nwin):
            lo, hi = w * wpairs, (w + 1) * wpairs
            ov = np.clip(np.minimum(p0[c] + cnt2[c], hi)
                         - np.maximum(p0[c], lo), 0, None)
            sizes_B[c, w] = ov.reshape(NG, tiles).sum(0)
    CB = sizes_B.max(0)                              # [nwin, tiles] shared
    sec = CB.sum(1)
    secpad = _roundup(sec, 128)
    wstart = np.zeros(nwin + 1, np.int64)
    wstart[1:] = np.cumsum(secpad)
    BS = int(wstart[-1])                             # total B pairs
    P_wt = np.cumsum(CB, axis=1) - CB + wstart[:nwin, None]

    bidx = np.zeros((n_cores, BS), np.int16)
    brow = np.full((n_cores, BS, 2), 255, np.int16)
    for c in range(n_cores):
        tot = int(cnt2[c].sum())
        if tot == 0:
            continue
        cums = np.cumsum(cnt2[c]) - cnt2[c]
        ap_all = (np.repeat(p0[c], cnt2[c])
                  + np.arange(tot) - np.repeat(cums, cnt2[c]))
        t_all = np.repeat(t_of_cell, cnt2[c])
        w_all = ap_all // wpairs
        key2 = w_all * tiles + t_all
        o2 = np.lexsort((ap_all, key2))
        k2, a2, t2, w2 = key2[o2], ap_all[o2], t_all[o2], w_all[o2]
        f2 = np.ones(tot, bool)
        f2[1:] = k2[1:] != k2[:-1]
        gs2 = np.maximum.accumulate(np.where(f2, np.arange(tot), 0))
        pos2 = np.arange(tot) - gs2
        bpos = P_wt[w2, t2] + pos2
        bidx[c, bpos] = (a2 - w2 * wpairs).astype(np.int16)
        brow[c, bpos, 0] = rowsA[c, a2 * 2]
        brow[c, bpos, 1] = rowsA[c, a2 * 2 + 1]

    # entries (block, w, t) + per-(w,t) entry spans
    entries = []
    ent_span = {}
    for w in range(nwin):
        for t in range(tiles):
            if CB[w, t] == 0:
                continue
            b0 = int(P_wt[w, t]) // 128
            b1 = (int(P_wt[w, t]) + int(CB[w, t]) + 127) // 128
            ent_span[(w, t)] = (len(entries), len(entries) + b1 - b0)
            entries.extend((b, w, t) for b in range(b0, b1))
    NENT = len(entries)

    metaI = np.full((n_cores, NENT, 128, 2), 255, np.int16)
    for e, (b, w, t) in enumerate(entries):
        lo = max(b * 128, int(P_wt[w, t]))
        hi = min((b + 1) * 128, int(P_wt[w, t]) + int(CB[w, t]))
        if hi > lo:
            metaI[:, e, lo - b * 128:hi - b * 128, :] = brow[:, lo:hi, :]
    meta = metaI.transpose(0, 2, 1, 3).astype(np.float16)

    # ---- segmentation ----
    # A calls: cut at chunk-section and window boundaries, then SEG slots.
    csec = [int(gstart[ci * K]) for ci in range(nchunk + 1)]
    bounds = sorted(set(
        csec + [w * wslots for w in range(nwin + 1) if w * wslots <= AS]
        + [AS]))
    k_of_group = np.tile(np.arange(K, dtype=np.int64), nchunk)
    k_of_slot = np.repeat(k_of_group, gcap)
    a_calls = []   # (slot0, ns, chunk_id, win, [k per tile])
    for lo, hi in zip(bounds[:-1], bounds[1:]):
        s = lo
        while s < hi:
            ns = min(SEG, hi - s)
            ks = [int(k_of_slot[s + 128 * j]) for j in range(ns // 128)]
            a_calls.append(
                (s, ns, int(np.searchsorted(csec, s, side="right") - 1),
                 s // wslots, ks))
            s += ns

    # B calls: per window section, SEG-pair chunks (128-multiples)
    b_calls = []   # (pair0, np_, w)
    for w in range(nwin):
        s = int(wstart[w])
        hi = int(wstart[w + 1])
        while s < hi:
            np_ = min(SEG, hi - s)
            b_calls.append((s, np_, w))
            s += np_
    call_lo = np.array([c[0] for c in b_calls])
    call_ent = [[] for _ in b_calls]
    for e, (b, w, t) in enumerate(entries):
        ci = int(np.searchsorted(call_lo, b * 128, side="right") - 1)
        assert b_calls[ci][0] <= b * 128 < b_calls[ci][0] + b_calls[ci][1]
        call_ent[ci].append(e)
    ne_max = max((len(x) for x in call_ent), default=0)

    first_w = {}
    last_w = {}
    for t in range(tiles):
        for w in range(nwin):
            if CB[w, t] > 0:
                if t not in first_w:
                    first_w[t] = w
                last_w[t] = w

    gidxA_w = np.zeros((n_cores, 128, AS // 16), np.int16)
    gidxB_w = np.zeros((n_cores, 128, BS // 16), np.int16)
    for c in range(n_cores):
        gidxA_w[c] = np.tile(gidxA[c].reshape(-1, 16).T, (8, 1))
        gidxB_w[c] = np.tile(bidx[c].reshape(-1, 16).T, (8, 1))

    plan = dict(
        K=K, tiles=tiles, nchunk=nchunk, nwin=nwin, AS=AS, BS=BS,
        chunk=chunk, wslots=wslots, rows_per_core=rows_per_core,
        a_calls=a_calls, b_calls=b_calls, entries=entries,
        ent_span=ent_span, call_ent=call_ent, ne_max=ne_max,
        first_w=first_w, last_w=last_w, NENT=NENT, n_out=int(n_out),
    )
    arrays = dict(gidxA=gidxA_w, gidxB=gidxB_w, meta=meta)
    return plan, arrays


def _build(plan, n_cores, ftab_rows):
    """Trace the Bass program."""
    _lazy()
    nc = bacc.Bacc("TRN2", target_bir_lowering=False, debug=False)

    K, tiles, nwin = plan["K"], plan["tiles"], plan["nwin"]
    AS, BS, NENT = plan["AS"], plan["BS"], plan["NENT"]
    ne_max = max(plan["ne_max"], 1)
    wslots = plan["wslots"]
    chunk = plan["chunk"]
    n_out = plan["n_out"]
    Cout = 64
    rows_pad = tiles * 128

    ftab = nc.dram_tensor("ftab", [ftab_rows, 128], F16, kind="ExternalInput")
    wt = nc.dram_tensor("wt", [128, K * Cout], F16, kind="ExternalInput")
    gidxA = nc.dram_tensor("gidxA", [128, AS // 16], I16, kind="ExternalInput")
    gidxB = nc.dram_tensor("gidxB", [128, BS // 16], I16, kind="ExternalInput")
    meta = nc.dram_tensor("meta", [128, NENT, 2], F16, kind="ExternalInput")
    iota2 = nc.dram_tensor("iota2", [128, 128, 2], F16,
                           kind="ExternalInput")
    gb = nc.dram_tensor("gb", [2, Cout], F32, kind="ExternalInput")
    atabs = [nc.dram_tensor(f"atab{w}", [wslots // 2, 128], F16)
             for w in range(nwin)]
    cc_in = nc.dram_tensor("cc_in", [2, Cout], F32)
    cc_out = nc.dram_tensor("cc_out", [2, Cout], F32, addr_space="Shared")
    y = nc.dram_tensor("y", [rows_pad, Cout], F32, kind="ExternalOutput")

    # slot-major write views of the contrib windows
    atv = [a[:, :].flatten().rearrange("(t p c) -> p t c", p=128, c=64)
           for a in atabs]

    entries = plan["entries"]
    ent_span = plan["ent_span"]
    call_ent = plan["call_ent"]
    first_w = plan["first_w"]
    last_w = plan["last_w"]
    t_order = []  # tiles in completion (emission) order
    for w in range(nwin):
        for t in range(tiles):
            if last_w.get(t) == w and (w, t) in ent_span:
                t_order.append(t)
    t_stat_first = t_order[0]
    t_stat_last = t_order[-1]
    ent_first = {}
    ent_last = {}
    for (w, t), (e0, e1) in ent_span.items():
        ent_first[(w, t)] = e0
        ent_last[(w, t)] = e1 - 1

    with tile.TileContext(nc) as tc:
        with (
            tc.tile_pool(name="const", bufs=1) as cpool,
            tc.tile_pool(name="agix", bufs=3) as agix,
            tc.tile_pool(name="ag", bufs=3) as agp,
            tc.tile_pool(name="aslab", bufs=3) as aslab,
            tc.tile_pool(name="bgix", bufs=3) as bgix,
            tc.tile_pool(name="bg", bufs=3) as bgp,
            tc.tile_pool(name="bmeta", bufs=3) as bmeta,
            tc.tile_pool(name="bs", bufs=3) as bspool,
            tc.tile_pool(name="slab", bufs=1) as slabpool,
        ):
            w_sb = cpool.tile([128, K * Cout], F16, tag="w")
            nc.sync.dma_start(out=w_sb[:, :], in_=wt[:, :])
            iota_sb = cpool.tile([128, 128, 2], F16, tag="iota")
            nc.sync.dma_start(out=iota_sb[:, :, :], in_=iota2[:, :, :])
            out_slab = slabpool.tile([128, tiles, Cout], F32, tag="slab")

            a_by_w = [[] for _ in range(nwin)]
            for call in plan["a_calls"]:
                a_by_w[call[3]].append(call)
            b_by_w = [[] for _ in range(nwin)]
            for ci, call in enumerate(plan["b_calls"]):
                b_by_w[call[2]].append((ci, call))

            psum_of = {}
            apsum, bpsum = [], []

            ABATCH = 8

            def a_batches(w):
                calls = a_by_w[w]
                outs = []
                for i0 in range(0, len(calls), ABATCH):
                    outs.append(_mk_a(calls[i0:i0 + ABATCH], w, i0))
                return outs

            def _mk_a(batch, w, i0):
                def go():
                    bs0 = batch[0][0]
                    bs1 = batch[-1][0] + batch[-1][1]
                    gib = agix.tile([128, ABATCH * SEG // 16], I16, tag="agi")
                    ldq = nc.scalar if (i0 // ABATCH) % 2 else nc.sync
                    ldq.dma_start(
                        out=gib[:, :(bs1 - bs0) // 16],
                        in_=gidxA[:, bs0 // 16:bs1 // 16])
                    for (s0, ns, ch, _w, ks) in batch:
                        nt = ns // 128
                        g = agp.tile([128, 1, SEG], F16, tag="ag")
                        nc.gpsimd.dma_gather(
                            out_ap=g[:, :, :ns],
                            in_ap=ftab[ch * chunk:(ch + 1) * chunk, :],
                            idxs_ap=gib[:, (s0 - bs0) // 16:
                                        (s0 - bs0 + ns) // 16],
                            num_idxs=ns,
                            num_idxs_reg=ns,
                            elem_size=128,
                            transpose=True,
                        )
                        ps = apsum[0].tile(
                            [128, SEG // 128, Cout], F32, tag="aps")
                        for j in range(nt):
                            nc.tensor.matmul(
                                out=ps[:, j, :],
                                lhsT=g[:, 0, j * 128:(j + 1) * 128],
                                rhs=w_sb[:, ks[j] * Cout:
                                         (ks[j] + 1) * Cout],
                                start=True, stop=True,
                            )
                        sl = aslab.tile(
                            [128, SEG // 128, Cout], F16, tag="asl")
                        nc.scalar.activation(
                            out=sl[:, :nt, :], in_=ps[:, :nt, :],
                            func=mybir.ActivationFunctionType.Copy)
                        t0 = (s0 - w * wslots) // 128
                        wq = nc.sync if (s0 // SEG) % 2 else nc.scalar
                        wq.dma_start(
                            out=atv[w][:, t0:t0 + nt, :], in_=sl[:, :nt, :])
                return go

            BBATCH = 8

            def b_batches(w):
                calls = b_by_w[w]
                outs = []
                for i0 in range(0, len(calls), BBATCH):
                    outs.append(_mk_b(calls[i0:i0 + BBATCH], w, i0))
                return outs

            def _mk_b(bat, w, i0):
                def go():
                    bp0 = bat[0][1][0]
                    bp1 = bat[-1][1][0] + bat[-1][1][1]
                    gib = bgix.tile([128, BBATCH * SEG // 16], I16, tag="bgi")
                    ldq = nc.scalar if (i0 // BBATCH) % 2 else nc.sync
                    ldq.dma_start(
                        out=gib[:, :(bp1 - bp0) // 16],
                        in_=gidxB[:, bp0 // 16:bp1 // 16])
                    be_lo = call_ent[bat[0][0]][0]
                    be_hi = call_ent[bat[-1][0]][-1] + 1
                    mtb = bmeta.tile([128, BBATCH * ne_max, 2], F16, tag="bm")
                    nc.scalar.dma_start(
                        out=mtb[:, :be_hi - be_lo, :],
                        in_=meta[:, be_lo:be_hi, :])
                    for (ci, (pair0, np_, _w)) in bat:
                        nb = np_ // 128
                        g = bgp.tile([128, SEG // 128, 128], F16, tag="bg")
                        nc.gpsimd.dma_gather(
                            out_ap=g[:, :nb, :],
                            in_ap=atabs[w][:, :],
                            idxs_ap=gib[:, (pair0 - bp0) // 16:
                                        (pair0 - bp0 + np_) // 16],
                            num_idxs=np_,
                            num_idxs_reg=np_,
                            elem_size=128,
                        )
                        ents = call_ent[ci]
                        if not ents:
                            continue
                        ne = len(ents)
                        e_lo = ents[0]
                        st = bspool.tile(
                            [128, 128, ne_max, 2], F16, tag="bsl")
                        nc.vector.tensor_tensor(
                            out=st[:, :, :ne, :],
                            in0=mtb[:, e_lo - be_lo:e_lo - be_lo + ne, :]
                            .unsqueeze(1).broadcast_to([128, 128, ne, 2]),
                            in1=iota_sb[:, :, :].unsqueeze(2)
                            .broadcast_to([128, 128, ne, 2]),
                            op=mybir.AluOpType.is_equal,
                        )
                        for e in ents:
                            b, we, t = entries[e]
                            tg = t // 4
                            if (we, tg) not in psum_of:
                                psum_of[(we, tg)] = bpsum[0].tile(
                                    [128, 4, Cout], F32, tag="bps",
                                    name=f"bps_{we}_{tg}")
                            ps = psum_of[(we, tg)]
                            last = e == ent_last[(we, t)]
                            for eo in range(2):
                                nc.tensor.matmul(
                                    out=ps[:, t % 4, :],
                                    lhsT=st[:, :, e - e_lo, eo],
                                    rhs=g[:, b - pair0 // 128,
                                          eo * 64:(eo + 1) * 64],
                                    start=(e == ent_first[(we, t)]
                                           and eo == 0),
                                    stop=(last and eo == 1),
                                )
                            if last and t == grp_last[(we, tg)]:
                                t0g, t1g = grp_span[(we, tg)]
                                nsl = t1g - t0g + 1
                                sl_ = out_slab[:, t0g:t1g + 1, :]
                                ps_ = ps[:, t0g - tg * 4:
                                         t0g - tg * 4 + nsl, :]
                                if first_w[t0g] == we:
                                    nc.vector.tensor_copy(
                                        out=sl_, in_=ps_)
                                else:
                                    nc.vector.tensor_tensor(
                                        out=sl_, in0=sl_, in1=ps_,
                                        op=mybir.AluOpType.add)
                                del psum_of[(we, tg)]
                                if last_w[t0g] == we:
                                    sqt = aslab.tile(
                                        [128, 4, Cout], F32, tag="sq",
                                        name=f"sq_{tg}")
                                    nc.vector.tensor_tensor(
                                        out=sqt[:, :nsl, :],
                                        in0=sl_, in1=sl_,
                                        op=mybir.AluOpType.mult)
                                    nc.tensor.matmul(
                                        out=stat_ps[:, 0:256],
                                        lhsT=ones1[:, :],
                                        rhs=out_slab[:, tg * 4:tg * 4 + 4, :],
                                        start=(tg == tg_stat_first),
                                        stop=(tg == tg_stat_last))
                                    nc.tensor.matmul(
                                        out=stat_ps[:, 256:512],
                                        lhsT=ones1[:, :],
                                        rhs=sq4_of(sqt, tg),
                                        start=(tg == tg_stat_first),
                                        stop=(tg == tg_stat_last))
                return go

            with (
                tc.tile_pool(name="apsum", bufs=3, space="PSUM") as apsum_,
                tc.tile_pool(name="bpsum", bufs=4, space="PSUM") as bpsum_,
                tc.tile_pool(name="spsum", bufs=1, space="PSUM") as spsum_,
            ):
                apsum.append(apsum_)
                bpsum.append(bpsum_)
                stat_ps = spsum_.tile([1, 512], F32, tag="stat")
                ones1 = cpool.tile([128, 1], F32, tag="ones1")
                nc.vector.memset(ones1[:, :], 1.0)
                for go in a_batches(0):
                    go()
                for w in range(1, nwin):
                    A, B = a_batches(w), b_batches(w - 1)
                    na, nb = len(A), len(B)
                    ia = ib = 0
                    while ia < na or ib < nb:
                        if ia < na and (ib >= nb or ia * nb <= ib * na):
                            A[ia]()
                            ia += 1
                        else:
                            B[ib]()
                            ib += 1
                for go in b_batches(nwin - 1):
                    go()
                st0 = cpool.tile([1, Cout], F32, tag="st0")
                st1 = cpool.tile([1, Cout], F32, tag="st1")
                nc.vector.tensor_copy(out=st0[:, :], in_=stat_ps[:, 0:64])
                nc.vector.tensor_copy(out=st1[:, :], in_=stat_ps[:, 64:128])
                nc.sync.dma_start(out=cc_in[0:1, :], in_=st0[:, :])
                nc.sync.dma_start(out=cc_in[1:2, :], in_=st1[:, :])
                nc.gpsimd.collective_compute(
                    "AllReduce",
                    mybir.AluOpType.add,
                    ins=[cc_in[:, :]],
                    outs=[cc_out[:, :]],
                    replica_groups=[list(range(n_cores))],
                )

            # ---- BN + ReLU ----
            with (
                tc.tile_pool(name="bn", bufs=4) as bnp,
                tc.tile_pool(name="bnps", bufs=2, space="PSUM") as bnps,
            ):
                gs0 = bnp.tile([1, Cout], F32, tag="gs0")
                gs1 = bnp.tile([1, Cout], F32, tag="gs1")
                nc.sync.dma_start(out=gs0[:, :], in_=cc_out[0:1, :])
                nc.sync.dma_start(out=gs1[:, :], in_=cc_out[1:2, :])
                gam_t = bnp.tile([1, Cout], F32, tag="gam")
                bet_t = bnp.tile([1, Cout], F32, tag="bet")
                nc.sync.dma_start(out=gam_t[:, :], in_=gb[0:1, :])
                nc.sync.dma_start(out=bet_t[:, :], in_=gb[1:2, :])

                inv_n = 1.0 / float(n_out)
                mean_t = bnp.tile([1, Cout], F32, tag="mean")
                ex2_t = bnp.tile([1, Cout], F32, tag="ex2")
                var_t = bnp.tile([1, Cout], F32, tag="var")
                sd_t = bnp.tile([1, Cout], F32, tag="sd")
                rs_t = bnp.tile([1, Cout], F32, tag="rs")
                a_t = bnp.tile([1, Cout], F32, tag="a")
                b_t = bnp.tile([1, Cout], F32, tag="b")
                nc.vector.tensor_scalar_mul(mean_t[:, :], gs0[:, :], inv_n)
                nc.vector.tensor_scalar_mul(ex2_t[:, :], gs1[:, :], inv_n)
                nc.vector.tensor_tensor(
                    out=var_t[:, :], in0=mean_t[:, :], in1=mean_t[:, :],
                    op=mybir.AluOpType.mult)
                nc.vector.tensor_tensor(
                    out=var_t[:, :], in0=ex2_t[:, :], in1=var_t[:, :],
                    op=mybir.AluOpType.subtract)
                nc.vector.tensor_scalar_add(var_t[:, :], var_t[:, :], BN_EPS)
                nc.scalar.activation(
                    out=sd_t[:, :], in_=var_t[:, :],
                    func=mybir.ActivationFunctionType.Sqrt)
                nc.vector.reciprocal(out=rs_t[:, :], in_=sd_t[:, :])
                nc.vector.tensor_tensor(
                    out=a_t[:, :], in0=gam_t[:, :], in1=rs_t[:, :],
                    op=mybir.AluOpType.mult)
                nc.vector.tensor_tensor(
                    out=b_t[:, :], in0=mean_t[:, :], in1=a_t[:, :],
                    op=mybir.AluOpType.mult)
                nc.vector.tensor_tensor(
                    out=b_t[:, :], in0=bet_t[:, :], in1=b_t[:, :],
                    op=mybir.AluOpType.subtract)
                ones_row = bnp.tile([1, 128], F32, tag="ones_row")
                nc.vector.memset(ones_row[:, :], 1.0)
                a_full = bnp.tile([128, Cout], F32, tag="afull")
                b_full = bnp.tile([128, Cout], F32, tag="bfull")
                ab_ps = bnps.tile([128, Cout], F32, tag="abps")
                nc.tensor.matmul(
                    out=ab_ps[:, :], lhsT=ones_row[:, :], rhs=a_t[:, :],
                    start=True, stop=True)
                nc.vector.tensor_copy(out=a_full[:, :], in_=ab_ps[:, :])
                nc.tensor.matmul(
                    out=ab_ps[:, :], lhsT=ones_row[:, :], rhs=b_t[:, :],
                    start=True, stop=True)
                nc.vector.tensor_copy(out=b_full[:, :], in_=ab_ps[:, :])
                # batched normalize + relu over the whole slab
                nc.vector.tensor_tensor(
                    out=out_slab[:, :, :], in0=out_slab[:, :, :],
                    in1=a_full[:, :].unsqueeze(1)
                    .broadcast_to([128, tiles, Cout]),
                    op=mybir.AluOpType.mult)
                nc.vector.tensor_tensor(
                    out=out_slab[:, :, :], in0=out_slab[:, :, :],
                    in1=b_full[:, :].unsqueeze(1)
                    .broadcast_to([128, tiles, Cout]),
                    op=mybir.AluOpType.add)
                nc.scalar.activation(
                    out=out_slab[:, :, :], in_=out_slab[:, :, :],
                    func=mybir.ActivationFunctionType.Relu)
                yv = y[:, :].flatten().rearrange(
                    "(t p c) -> p t c", p=128, c=64)
                nc.sync.dma_start(out=yv, in_=out_slab[:, :, :])

    nc.compile()
    return nc


def _prepare(feats, W, gamma, beta, in_map, out_map, n_out,
             n_cores=8, dup_safe=False, expand=1):
    """Host prep shared by kernel() and tests. Returns (nc, in_maps, plan)."""
    _lazy()
    n_out = int(n_out)
    K, Cin, Cout = W.shape
    assert Cin == 64 and Cout == 64
    rows_per_core = n_out // n_cores
    assert rows_per_core * n_cores == n_out

    in_map = np.asarray(in_map, dtype=np.int64)
    out_map = np.asarray(out_map, dtype=np.int64)
    feats = np.asarray(feats, dtype=np.float32)
    W = np.asarray(W, dtype=np.float32)

    plan, arrays = _plan(in_map, out_map, n_out, n_cores, rows_per_core,
                         CHUNK, WSLOTS)

    ftab_rows = _roundup(feats.shape[0], CHUNK)
    ftab = np.zeros((ftab_rows, 128), dtype=np.float16)
    ftab[:feats.shape[0], :64] = feats.astype(np.float16)
    wt = np.zeros((128, K * 64), dtype=np.float16)
    wt[:64, :] = W.transpose(1, 0, 2).reshape(64, K * 64).astype(np.float16)
    gb = np.stack([np.asarray(gamma, np.float32),
                   np.asarray(beta, np.float32)])
    iota2 = np.broadcast_to(np.arange(128, dtype=np.float32)[None, :, None],
                            (128, 128, 2)).astype(np.float16)

    nc = _build(plan, n_cores, ftab_rows)
    in_maps = [
        dict(ftab=ftab, wt=wt, gb=gb, iota2=iota2,
             gidxA=arrays["gidxA"][c], gidxB=arrays["gidxB"][c],
             meta=arrays["meta"][c])
        for c in range(n_cores)
    ]
    return nc, in_maps, plan


def kernel(feats, W, gamma, beta, in_map, out_map, n_out):
    _lazy()
    from concourse.bass_utils import run_bass_kernel_spmd

    n_cores = 8
    nc, in_maps, plan = _prepare(
        feats, W, gamma, beta, in_map, out_map, n_out, n_cores)
    res = run_bass_kernel_spmd(nc, in_maps, list(range(n_cores)))
    rows = plan["rows_per_core"]
    out = np.concatenate(
        [res.results[c]["y"][:rows] for c in range(n_cores)], axis=0)
    return out.astype(np.float32)


# revision 17
# speedup vs baseline: 2.4734x; 1.0708x over previous
"""Trainium2 Bass kernel for nn_BasicDeconvolutionBlock (two-phase design).

Reference computation:
    gathered = feats[in_map]                         # [K, M, Cin]
    contrib  = einsum('kmc,kcd->kmd', gathered, W)   # [K, M, Cout]
    out      = zeros([n_out, Cout]).at[out_map].add(contrib)
    y        = relu(batchnorm(out))                  # batch stats over n_out rows

Strategy (8 NeuronCores, SPMD, output-row sharding):
  Host routes each (k, m) pair to the core owning its output row
  (row blocks of n_out/8, ~169k pairs/core), orders the pairs by
  (feats-chunk, k, out-tile) with per-(chunk,k,out-tile) "cells" padded to
  even length, groups (chunk,k) padded to 128.

  Phase A (gather-GEMM): SWDGE dma_gather (transpose) pulls feats rows
  (fp16, 256B) channel-major; per-128-slot matmul against W[k] (fp16);
  PSUM -> fp16 slab (scalar engine Copy) -> contiguous HBM contrib table
  (slot-major 128B rows, in window tensors of 64k slots so phase B's
  int16 gather indices stay in range).  No scatter-add, no occurrence
  rounds.

  Phase B (gather-reduce): contrib rows are fetched in out-tile order as
  PAIRS (256B descriptors = 2 rows, halving descriptor count; cells are
  even-aligned so pairs never straddle cells).  One-hot S matrices
  ([128 pairs x 128 rows], fp16) are built on-chip with a single
  broadcast is_equal against an iota tile per gather call; matmul
  lhsT=S, rhs=gathered pair block accumulates the segmented scatter-add
  directly in PSUM per 128-row out-tile.  Tiles accumulate across the
  window passes into an SBUF fp32 slab.

  BN: per-tile ones-matmul row sums + sum of squares, [2,64] AllReduce
  across 8 cores, batched normalize + ReLU, output shard [25088,64] fp32.
"""

import numpy as np

BN_EPS = 1e-5
SEG = 896            # max descriptors per SWDGE call (Q7 ucode ring limit)
CHUNK = 32768        # int16 gather index range per feats chunk
WSLOTS = 65536       # contrib-table window: 32768 pairs of slots


def _lazy():
    global F32, F16, I16, mybir, bacc, tile
    import sys
    for p in ("/opt/trn_rl_repo",):
        if p not in sys.path:
            sys.path.insert(0, p)
    from concourse import bacc as _bacc, mybir as _mybir
    import concourse.tile as _tile
    mybir, bacc, tile = _mybir, _bacc, _tile
    F32 = mybir.dt.float32
    F16 = mybir.dt.float16
    I16 = mybir.dt.int16


def _roundup(x, m):
    return (x + m - 1) // m * m


def _plan(in_map, out_map, n_out, n_cores, rows_per_core, chunk, wslots):
    """Host-side routing. Returns a dict plan + per-core packed arrays."""
    K, M = in_map.shape
    tiles = _roundup(rows_per_core, 128) // 128
    wpairs = wslots // 2
    in_flat = np.asarray(in_map).ravel().astype(np.int64)
    out_flat = np.asarray(out_map).ravel().astype(np.int64)
    k_idx = np.repeat(np.arange(K, dtype=np.int64), M)
    core = out_flat // rows_per_core
    row_local = out_flat - core * rows_per_core
    t_idx = row_local >> 7
    r128 = row_local & 127
    chnk = in_flat // chunk
    idx_local = in_flat - chnk * chunk
    nchunk = int(chnk.max()) + 1
    NG = nchunk * K
    NCELLS = NG * tiles
    cell = (chnk * K + k_idx) * tiles + t_idx

    sizes = np.zeros((n_cores, NCELLS), np.int64)
    for c in range(n_cores):
        sizes[c] = np.bincount(cell[core == c], minlength=NCELLS)
    ce = (sizes + 1) // 2 * 2                       # cell sizes even-padded
    gsz = ce.reshape(n_cores, NG, tiles).sum(-1)
    gcap = _roundup(gsz.max(0), 128)                # [NG] shared
    gstart = np.zeros(NG + 1, np.int64)
    gstart[1:] = np.cumsum(gcap)
    AS = int(gstart[-1])                            # total A slots
    nwin = _roundup(AS, wslots) // wslots

    ce3 = ce.reshape(n_cores, NG, tiles)
    cstart = (np.cumsum(ce3, axis=2) - ce3
              + gstart[None, :NG, None])            # [cores, NG, tiles]
    cstart_f = cstart.reshape(n_cores, NCELLS)

    order = np.lexsort((cell, core))
    cell_s, core_s = cell[order], core[order]
    key = core_s * NCELLS + cell_s
    n = len(key)
    first = np.ones(n, bool)
    first[1:] = key[1:] != key[:-1]
    gs = np.maximum.accumulate(np.where(first, np.arange(n), 0))
    pos = np.arange(n) - gs
    Apos_s = cstart_f[core_s, cell_s] + pos

    gidxA = np.zeros((n_cores, AS), np.int16)
    rowsA = np.full((n_cores, AS), 255, np.int16)
    gidxA[core_s, Apos_s] = idx_local[order].astype(np.int16)
    rowsA[core_s, Apos_s] = r128[order].astype(np.int16)

    # ---- B stream ----
    p0 = cstart_f // 2
    cnt2 = ce.reshape(n_cores, NCELLS) // 2          # pairs per cell
    t_of_cell = np.tile(np.arange(tiles, dtype=np.int64), NG)

    sizes_B = np.zeros((n_cores, nwin, tiles), np.int64)
    for c in range(n_cores):
        for w in range(nwin):
            lo, hi = w * wpairs, (w + 1) * wpairs
            ov = np.clip(np.minimum(p0[c] + cnt2[c], hi)
                         - np.maximum(p0[c], lo), 0, None)
            sizes_B[c, w] = ov.reshape(NG, tiles).sum(0)
    CB = sizes_B.max(0)                              # [nwin, tiles] shared
    sec = CB.sum(1)
    secpad = _roundup(sec, 128)
    wstart = np.zeros(nwin + 1, np.int64)
    wstart[1:] = np.cumsum(secpad)
    BS = int(wstart[-1])                             # total B pairs
    P_wt = np.cumsum(CB, axis=1) - CB + wstart[:nwin, None]

    bidx = np.zeros((n_cores, BS), np.int16)
    brow = np.full((n_cores, BS, 2), 255, np.int16)
    for c in range(n_cores):
        tot = int(cnt2[c].sum())
        if tot == 0:
            continue
        cums = np.cumsum(cnt2[c]) - cnt2[c]
        ap_all = (np.repeat(p0[c], cnt2[c])
                  + np.arange(tot) - np.repeat(cums, cnt2[c]))
        t_all = np.repeat(t_of_cell, cnt2[c])
        w_all = ap_all // wpairs
        key2 = w_all * tiles + t_all
        o2 = np.lexsort((ap_all, key2))
        k2, a2, t2, w2 = key2[o2], ap_all[o2], t_all[o2], w_all[o2]
        f2 = np.ones(tot, bool)
        f2[1:] = k2[1:] != k2[:-1]
        gs2 = np.maximum.accumulate(np.where(f2, np.arange(tot), 0))
        pos2 = np.arange(tot) - gs2
        bpos = P_wt[w2, t2] + pos2
        bidx[c, bpos] = (a2 - w2 * wpairs).astype(np.int16)
        brow[c, bpos, 0] = rowsA[c, a2 * 2]
        brow[c, bpos, 1] = rowsA[c, a2 * 2 + 1]

    # entries (block, w, t) + per-(w,t) entry spans
    entries = []
    ent_span = {}
    for w in range(nwin):
        for t in range(tiles):
            if CB[w, t] == 0:
                continue
            b0 = int(P_wt[w, t]) // 128
            b1 = (int(P_wt[w, t]) + int(CB[w, t]) + 127) // 128
            ent_span[(w, t)] = (len(entries), len(entries) + b1 - b0)
            entries.extend((b, w, t) for b in range(b0, b1))
    NENT = len(entries)

    metaI = np.full((n_cores, NENT, 128, 2), 255, np.int16)
    for e, (b, w, t) in enumerate(entries):
        lo = max(b * 128, int(P_wt[w, t]))
        hi = min((b + 1) * 128, int(P_wt[w, t]) + int(CB[w, t]))
        if hi > lo:
            metaI[:, e, lo - b * 128:hi - b * 128, :] = brow[:, lo:hi, :]
    meta = metaI.transpose(0, 2, 1, 3).astype(np.float16)

    # ---- segmentation ----
    # A calls: cut at chunk-section and window boundaries, then SEG slots.
    csec = [int(gstart[ci * K]) for ci in range(nchunk + 1)]
    bounds = sorted(set(
        csec + [w * wslots for w in range(nwin + 1) if w * wslots <= AS]
        + [AS]))
    k_of_group = np.tile(np.arange(K, dtype=np.int64), nchunk)
    k_of_slot = np.repeat(k_of_group, gcap)
    a_calls = []   # (slot0, ns, chunk_id, win, [k per tile])
    for lo, hi in zip(bounds[:-1], bounds[1:]):
        s = lo
        while s < hi:
            ns = min(SEG, hi - s)
            ks = [int(k_of_slot[s + 128 * j]) for j in range(ns // 128)]
            a_calls.append(
                (s, ns, int(np.searchsorted(csec, s, side="right") - 1),
                 s // wslots, ks))
            s += ns

    # B calls: per window section, SEG-pair chunks (128-multiples)
    b_calls = []   # (pair0, np_, w)
    for w in range(nwin):
        s = int(wstart[w])
        hi = int(wstart[w + 1])
        while s < hi:
            np_ = min(SEG, hi - s)
            b_calls.append((s, np_, w))
            s += np_
    call_lo = np.array([c[0] for c in b_calls])
    call_ent = [[] for _ in b_calls]
    for e, (b, w, t) in enumerate(entries):
        ci = int(np.searchsorted(call_lo, b * 128, side="right") - 1)
        assert b_calls[ci][0] <= b * 128 < b_calls[ci][0] + b_calls[ci][1]
        call_ent[ci].append(e)
    ne_max = max((len(x) for x in call_ent), default=0)

    first_w = {}
    last_w = {}
    for t in range(tiles):
        for w in range(nwin):
            if CB[w, t] > 0:
                if t not in first_w:
                    first_w[t] = w
                last_w[t] = w

    gidxA_w = np.zeros((n_cores, 128, AS // 16), np.int16)
    gidxB_w = np.zeros((n_cores, 128, BS // 16), np.int16)
    for c in range(n_cores):
        gidxA_w[c] = np.tile(gidxA[c].reshape(-1, 16).T, (8, 1))
        gidxB_w[c] = np.tile(bidx[c].reshape(-1, 16).T, (8, 1))

    plan = dict(
        K=K, tiles=tiles, nchunk=nchunk, nwin=nwin, AS=AS, BS=BS,
        chunk=chunk, wslots=wslots, rows_per_core=rows_per_core,
        a_calls=a_calls, b_calls=b_calls, entries=entries,
        ent_span=ent_span, call_ent=call_ent, ne_max=ne_max,
        first_w=first_w, last_w=last_w, NENT=NENT, n_out=int(n_out),
    )
    arrays = dict(gidxA=gidxA_w, gidxB=gidxB_w, meta=meta)
    return plan, arrays


def _build(plan, n_cores, ftab_rows):
    """Trace the Bass program."""
    _lazy()
    nc = bacc.Bacc("TRN2", target_bir_lowering=False, debug=False)

    K, tiles, nwin = plan["K"], plan["tiles"], plan["nwin"]
    AS, BS, NENT = plan["AS"], plan["BS"], plan["NENT"]
    ne_max = max(plan["ne_max"], 1)
    wslots = plan["wslots"]
    chunk = plan["chunk"]
    n_out = plan["n_out"]
    Cout = 64
    rows_pad = tiles * 128

    ftab = nc.dram_tensor("ftab", [ftab_rows, 128], F16, kind="ExternalInput")
    wt = nc.dram_tensor("wt", [128, K * Cout], F16, kind="ExternalInput")
    gidxA = nc.dram_tensor("gidxA", [128, AS // 16], I16, kind="ExternalInput")
    gidxB = nc.dram_tensor("gidxB", [128, BS // 16], I16, kind="ExternalInput")
    meta = nc.dram_tensor("meta", [128, NENT, 2], F16, kind="ExternalInput")
    iota2 = nc.dram_tensor("iota2", [128, 128, 2], F16,
                           kind="ExternalInput")
    gb = nc.dram_tensor("gb", [2, Cout], F32, kind="ExternalInput")
    atabs = [nc.dram_tensor(f"atab{w}", [wslots // 2, 128], F16)
             for w in range(nwin)]
    cc_in = nc.dram_tensor("cc_in", [2, Cout], F32)
    cc_out = nc.dram_tensor("cc_out", [2, Cout], F32, addr_space="Shared")
    y = nc.dram_tensor("y", [rows_pad, Cout], F16, kind="ExternalOutput")

    # slot-major write views of the contrib windows
    atv = [a[:, :].flatten().rearrange("(t p c) -> p t c", p=128, c=64)
           for a in atabs]

    entries = plan["entries"]
    ent_span = plan["ent_span"]
    call_ent = plan["call_ent"]
    first_w = plan["first_w"]
    last_w = plan["last_w"]
    # full 4-tile groups: every (w, t) cell must be populated
    assert tiles % 4 == 0
    for w in range(nwin):
        for t in range(tiles):
            assert (w, t) in ent_span, (w, t)
    assert all(v == 0 for v in first_w.values())
    assert all(v == nwin - 1 for v in last_w.values())
    tg_stat_first = 0
    tg_stat_last = tiles // 4 - 1
    ent_first = {}
    ent_last = {}
    for (w, t), (e0, e1) in ent_span.items():
        ent_first[(w, t)] = e0
        ent_last[(w, t)] = e1 - 1

    with tile.TileContext(nc) as tc:
        with (
            tc.tile_pool(name="const", bufs=1) as cpool,
            tc.tile_pool(name="agix", bufs=3) as agix,
            tc.tile_pool(name="ag", bufs=3) as agp,
            tc.tile_pool(name="aslab", bufs=3) as aslab,
            tc.tile_pool(name="bgix", bufs=3) as bgix,
            tc.tile_pool(name="bg", bufs=3) as bgp,
            tc.tile_pool(name="bmeta", bufs=3) as bmeta,
            tc.tile_pool(name="bs", bufs=3) as bspool,
            tc.tile_pool(name="slab", bufs=1) as slabpool,
        ):
            w_sb = cpool.tile([128, K * Cout], F16, tag="w")
            nc.sync.dma_start(out=w_sb[:, :], in_=wt[:, :])
            iota_sb = cpool.tile([128, 128, 2], F16, tag="iota")
            nc.sync.dma_start(out=iota_sb[:, :, :], in_=iota2[:, :, :])
            out_slab = slabpool.tile([128, tiles, Cout], F32, tag="slab")

            a_by_w = [[] for _ in range(nwin)]
            for call in plan["a_calls"]:
                a_by_w[call[3]].append(call)
            b_by_w = [[] for _ in range(nwin)]
            for ci, call in enumerate(plan["b_calls"]):
                b_by_w[call[2]].append((ci, call))

            psum_of = {}
            apsum, bpsum = [], []

            ABATCH = 8

            def a_batches(w):
                calls = a_by_w[w]
                outs = []
                for i0 in range(0, len(calls), ABATCH):
                    outs.append(_mk_a(calls[i0:i0 + ABATCH], w, i0))
                return outs

            def _mk_a(batch, w, i0):
                def go():
                    bs0 = batch[0][0]
                    bs1 = batch[-1][0] + batch[-1][1]
                    gib = agix.tile([128, ABATCH * SEG // 16], I16, tag="agi")
                    ldq = nc.scalar if (i0 // ABATCH) % 2 else nc.sync
                    ldq.dma_start(
                        out=gib[:, :(bs1 - bs0) // 16],
                        in_=gidxA[:, bs0 // 16:bs1 // 16])
                    for (s0, ns, ch, _w, ks) in batch:
                        nt = ns // 128
                        g = agp.tile([128, 1, SEG], F16, tag="ag")
                        nc.gpsimd.dma_gather(
                            out_ap=g[:, :, :ns],
                            in_ap=ftab[ch * chunk:(ch + 1) * chunk, :],
                            idxs_ap=gib[:, (s0 - bs0) // 16:
                                        (s0 - bs0 + ns) // 16],
                            num_idxs=ns,
                            num_idxs_reg=ns,
                            elem_size=128,
                            transpose=True,
                        )
                        ps = apsum[0].tile(
                            [128, SEG // 128, Cout], F32, tag="aps")
                        for j in range(nt):
                            nc.tensor.matmul(
                                out=ps[:, j, :],
                                lhsT=g[:, 0, j * 128:(j + 1) * 128],
                                rhs=w_sb[:, ks[j] * Cout:
                                         (ks[j] + 1) * Cout],
                                start=True, stop=True,
                            )
                        sl = aslab.tile(
                            [128, SEG // 128, Cout], F16, tag="asl")
                        nc.scalar.activation(
                            out=sl[:, :nt, :], in_=ps[:, :nt, :],
                            func=mybir.ActivationFunctionType.Copy)
                        t0 = (s0 - w * wslots) // 128
                        wq = nc.sync if (s0 // SEG) % 2 else nc.scalar
                        wq.dma_start(
                            out=atv[w][:, t0:t0 + nt, :], in_=sl[:, :nt, :])
                return go

            BBATCH = 8

            def b_batches(w):
                calls = b_by_w[w]
                outs = []
                for i0 in range(0, len(calls), BBATCH):
                    outs.append(_mk_b(calls[i0:i0 + BBATCH], w, i0))
                return outs

            def _mk_b(bat, w, i0):
                def go():
                    bp0 = bat[0][1][0]
                    bp1 = bat[-1][1][0] + bat[-1][1][1]
                    gib = bgix.tile([128, BBATCH * SEG // 16], I16, tag="bgi")
                    ldq = nc.scalar if (i0 // BBATCH) % 2 else nc.sync
                    ldq.dma_start(
                        out=gib[:, :(bp1 - bp0) // 16],
                        in_=gidxB[:, bp0 // 16:bp1 // 16])
                    be_lo = call_ent[bat[0][0]][0]
                    be_hi = call_ent[bat[-1][0]][-1] + 1
                    mtb = bmeta.tile([128, BBATCH * ne_max, 2], F16, tag="bm")
                    nc.scalar.dma_start(
                        out=mtb[:, :be_hi - be_lo, :],
                        in_=meta[:, be_lo:be_hi, :])
                    for (ci, (pair0, np_, _w)) in bat:
                        nb = np_ // 128
                        g = bgp.tile([128, SEG // 128, 128], F16, tag="bg")
                        nc.gpsimd.dma_gather(
                            out_ap=g[:, :nb, :],
                            in_ap=atabs[w][:, :],
                            idxs_ap=gib[:, (pair0 - bp0) // 16:
                                        (pair0 - bp0 + np_) // 16],
                            num_idxs=np_,
                            num_idxs_reg=np_,
                            elem_size=128,
                        )
                        ents = call_ent[ci]
                        if not ents:
                            continue
                        ne = len(ents)
                        e_lo = ents[0]
                        st = bspool.tile(
                            [128, 128, ne_max, 2], F16, tag="bsl")
                        nc.vector.tensor_tensor(
                            out=st[:, :, :ne, :],
                            in0=mtb[:, e_lo - be_lo:e_lo - be_lo + ne, :]
                            .unsqueeze(1).broadcast_to([128, 128, ne, 2]),
                            in1=iota_sb[:, :, :].unsqueeze(2)
                            .broadcast_to([128, 128, ne, 2]),
                            op=mybir.AluOpType.is_equal,
                        )
                        for e in ents:
                            b, we, t = entries[e]
                            tg = t // 4
                            if (we, tg) not in psum_of:
                                psum_of[(we, tg)] = bpsum[0].tile(
                                    [128, 4, Cout], F32, tag="bps",
                                    name=f"bps_{we}_{tg}")
                            ps = psum_of[(we, tg)]
                            last = e == ent_last[(we, t)]
                            for eo in range(2):
                                nc.tensor.matmul(
                                    out=ps[:, t % 4, :],
                                    lhsT=st[:, :, e - e_lo, eo],
                                    rhs=g[:, b - pair0 // 128,
                                          eo * 64:(eo + 1) * 64],
                                    start=(e == ent_first[(we, t)]
                                           and eo == 0),
                                    stop=(last and eo == 1),
                                )
                            if last and t == tg * 4 + 3:
                                sl_ = out_slab[:, tg * 4:tg * 4 + 4, :]
                                if we == 0:
                                    nc.vector.tensor_copy(
                                        out=sl_, in_=ps[:, :, :])
                                else:
                                    nc.vector.tensor_tensor(
                                        out=sl_, in0=sl_, in1=ps[:, :, :],
                                        op=mybir.AluOpType.add)
                                del psum_of[(we, tg)]
                                if we == nwin - 1:
                                    sqt = aslab.tile(
                                        [128, 4, Cout], F32, tag="sq",
                                        name=f"sq_{tg}")
                                    nc.vector.tensor_tensor(
                                        out=sqt[:, :, :],
                                        in0=sl_, in1=sl_,
                                        op=mybir.AluOpType.mult)
                                    nc.tensor.matmul(
                                        out=stat_ps[:, 0:256],
                                        lhsT=ones1[:, :],
                                        rhs=sl_,
                                        start=(tg == tg_stat_first),
                                        stop=(tg == tg_stat_last))
                                    nc.tensor.matmul(
                                        out=stat_ps[:, 256:512],
                                        lhsT=ones1[:, :],
                                        rhs=sqt[:, :, :],
                                        start=(tg == tg_stat_first),
                                        stop=(tg == tg_stat_last))
                return go

            with (
                tc.tile_pool(name="apsum", bufs=3, space="PSUM") as apsum_,
                tc.tile_pool(name="bpsum", bufs=4, space="PSUM") as bpsum_,
                tc.tile_pool(name="spsum", bufs=1, space="PSUM") as spsum_,
            ):
                apsum.append(apsum_)
                bpsum.append(bpsum_)
                stat_ps = spsum_.tile([1, 512], F32, tag="stat")
                ones1 = cpool.tile([128, 1], F32, tag="ones1")
                nc.vector.memset(ones1[:, :], 1.0)
                for go in a_batches(0):
                    go()
                for w in range(1, nwin):
                    A, B = a_batches(w), b_batches(w - 1)
                    na, nb = len(A), len(B)
                    ia = ib = 0
                    while ia < na or ib < nb:
                        if ia < na and (ib >= nb or ia * nb <= ib * na):
                            A[ia]()
                            ia += 1
                        else:
                            B[ib]()
                            ib += 1
                for go in b_batches(nwin - 1):
                    go()
                stf = cpool.tile([1, 512], F32, tag="stf")
                nc.vector.tensor_copy(out=stf[:, :], in_=stat_ps[:, :])
                # fold 4 tile-columns into one (sum region and sq region)
                st0 = cpool.tile([1, Cout], F32, tag="st0")
                st1 = cpool.tile([1, Cout], F32, tag="st1")
                nc.vector.tensor_copy(out=st0[:, :], in_=stf[:, 0:64])
                for jj in range(1, 4):
                    nc.vector.tensor_tensor(
                        out=st0[:, :], in0=st0[:, :],
                        in1=stf[:, jj * 64:(jj + 1) * 64],
                        op=mybir.AluOpType.add)
                nc.vector.tensor_copy(out=st1[:, :], in_=stf[:, 256:320])
                for jj in range(1, 4):
                    nc.vector.tensor_tensor(
                        out=st1[:, :], in0=st1[:, :],
                        in1=stf[:, 256 + jj * 64:256 + (jj + 1) * 64],
                        op=mybir.AluOpType.add)
                nc.sync.dma_start(out=cc_in[0:1, :], in_=st0[:, :])
                nc.sync.dma_start(out=cc_in[1:2, :], in_=st1[:, :])
                nc.gpsimd.collective_compute(
                    "AllReduce",
                    mybir.AluOpType.add,
                    ins=[cc_in[:, :]],
                    outs=[cc_out[:, :]],
                    replica_groups=[list(range(n_cores))],
                )

            # ---- BN + ReLU ----
            with (
                tc.tile_pool(name="bn", bufs=4) as bnp,
                tc.tile_pool(name="bnps", bufs=2, space="PSUM") as bnps,
            ):
                gs0 = bnp.tile([1, Cout], F32, tag="gs0")
                gs1 = bnp.tile([1, Cout], F32, tag="gs1")
                nc.sync.dma_start(out=gs0[:, :], in_=cc_out[0:1, :])
                nc.sync.dma_start(out=gs1[:, :], in_=cc_out[1:2, :])
                gam_t = bnp.tile([1, Cout], F32, tag="gam")
                bet_t = bnp.tile([1, Cout], F32, tag="bet")
                nc.sync.dma_start(out=gam_t[:, :], in_=gb[0:1, :])
                nc.sync.dma_start(out=bet_t[:, :], in_=gb[1:2, :])

                inv_n = 1.0 / float(n_out)
                mean_t = bnp.tile([1, Cout], F32, tag="mean")
                ex2_t = bnp.tile([1, Cout], F32, tag="ex2")
                var_t = bnp.tile([1, Cout], F32, tag="var")
                sd_t = bnp.tile([1, Cout], F32, tag="sd")
                rs_t = bnp.tile([1, Cout], F32, tag="rs")
                a_t = bnp.tile([1, Cout], F32, tag="a")
                b_t = bnp.tile([1, Cout], F32, tag="b")
                nc.vector.tensor_scalar_mul(mean_t[:, :], gs0[:, :], inv_n)
                nc.vector.tensor_scalar_mul(ex2_t[:, :], gs1[:, :], inv_n)
                nc.vector.tensor_tensor(
                    out=var_t[:, :], in0=mean_t[:, :], in1=mean_t[:, :],
                    op=mybir.AluOpType.mult)
                nc.vector.tensor_tensor(
                    out=var_t[:, :], in0=ex2_t[:, :], in1=var_t[:, :],
                    op=mybir.AluOpType.subtract)
                nc.vector.tensor_scalar_add(var_t[:, :], var_t[:, :], BN_EPS)
                nc.scalar.activation(
                    out=sd_t[:, :], in_=var_t[:, :],
                    func=mybir.ActivationFunctionType.Sqrt)
                nc.vector.reciprocal(out=rs_t[:, :], in_=sd_t[:, :])
                nc.vector.tensor_tensor(
                    out=a_t[:, :], in0=gam_t[:, :], in1=rs_t[:, :],
                    op=mybir.AluOpType.mult)
                nc.vector.tensor_tensor(
                    out=b_t[:, :], in0=mean_t[:, :], in1=a_t[:, :],
                    op=mybir.AluOpType.mult)
                nc.vector.tensor_tensor(
                    out=b_t[:, :], in0=bet_t[:, :], in1=b_t[:, :],
                    op=mybir.AluOpType.subtract)
                ones_row = bnp.tile([1, 128], F32, tag="ones_row")
                nc.vector.memset(ones_row[:, :], 1.0)
                a_full = bnp.tile([128, Cout], F32, tag="afull")
                b_full = bnp.tile([128, Cout], F32, tag="bfull")
                ab_ps = bnps.tile([128, Cout], F32, tag="abps")
                nc.tensor.matmul(
                    out=ab_ps[:, :], lhsT=ones_row[:, :], rhs=a_t[:, :],
                    start=True, stop=True)
                nc.vector.tensor_copy(out=a_full[:, :], in_=ab_ps[:, :])
                nc.tensor.matmul(
                    out=ab_ps[:, :], lhsT=ones_row[:, :], rhs=b_t[:, :],
                    start=True, stop=True)
                nc.vector.tensor_copy(out=b_full[:, :], in_=ab_ps[:, :])
                # batched normalize + relu + write, pipelined in chunks
                yv = y[:, :].flatten().rearrange(
                    "(t p c) -> p t c", p=128, c=64)
                nchk = 4
                step = tiles // nchk
                for ci_ in range(nchk):
                    tlo = ci_ * step
                    thi = tiles if ci_ == nchk - 1 else (ci_ + 1) * step
                    nn = thi - tlo
                    sl_ = out_slab[:, tlo:thi, :]
                    nc.vector.tensor_tensor(
                        out=sl_, in0=sl_,
                        in1=a_full[:, :].unsqueeze(1)
                        .broadcast_to([128, nn, Cout]),
                        op=mybir.AluOpType.mult)
                    nc.vector.tensor_tensor(
                        out=sl_, in0=sl_,
                        in1=b_full[:, :].unsqueeze(1)
                        .broadcast_to([128, nn, Cout]),
                        op=mybir.AluOpType.add)
                    y16 = bnp.tile([128, step, Cout], F16, tag="y16")
                    nc.scalar.activation(
                        out=y16[:, :nn, :], in_=sl_,
                        func=mybir.ActivationFunctionType.Relu)
                    wq = nc.sync if ci_ % 2 else nc.scalar
                    wq.dma_start(out=yv[:, tlo:thi, :], in_=y16[:, :nn, :])

    nc.compile()
    return nc


def _prepare(feats, W, gamma, beta, in_map, out_map, n_out,
             n_cores=8, dup_safe=False, expand=1):
    """Host prep shared by kernel() and tests. Returns (nc, in_maps, plan)."""
    _lazy()
    n_out = int(n_out)
    K, Cin, Cout = W.shape
    assert Cin == 64 and Cout == 64
    rows_per_core = n_out // n_cores
    assert rows_per_core * n_cores == n_out

    in_map = np.asarray(in_map, dtype=np.int64)
    out_map = np.asarray(out_map, dtype=np.int64)
    feats = np.asarray(feats, dtype=np.float32)
    W = np.asarray(W, dtype=np.float32)

    plan, arrays = _plan(in_map, out_map, n_out, n_cores, rows_per_core,
                         CHUNK, WSLOTS)

    ftab_rows = _roundup(feats.shape[0], CHUNK)
    ftab = np.zeros((ftab_rows, 128), dtype=np.float16)
    ftab[:feats.shape[0], :64] = feats.astype(np.float16)
    wt = np.zeros((128, K * 64), dtype=np.float16)
    wt[:64, :] = W.transpose(1, 0, 2).reshape(64, K * 64).astype(np.float16)
    gb = np.stack([np.asarray(gamma, np.float32),
                   np.asarray(beta, np.float32)])
    iota2 = np.broadcast_to(np.arange(128, dtype=np.float32)[None, :, None],
                            (128, 128, 2)).astype(np.float16)

    nc = _build(plan, n_cores, ftab_rows)
    in_maps = [
        dict(ftab=ftab, wt=wt, gb=gb, iota2=iota2,
             gidxA=arrays["gidxA"][c], gidxB=arrays["gidxB"][c],
             meta=arrays["meta"][c])
        for c in range(n_cores)
    ]
    return nc, in_maps, plan


def kernel(feats, W, gamma, beta, in_map, out_map, n_out):
    _lazy()
    from concourse.bass_utils import run_bass_kernel_spmd

    n_cores = 8
    nc, in_maps, plan = _prepare(
        feats, W, gamma, beta, in_map, out_map, n_out, n_cores)
    res = run_bass_kernel_spmd(nc, in_maps, list(range(n_cores)))
    rows = plan["rows_per_core"]
    out = np.concatenate(
        [res.results[c]["y"][:rows] for c in range(n_cores)], axis=0)
    return out.astype(np.float32)


# revision 23
# speedup vs baseline: 2.6249x; 1.0612x over previous
"""Trainium2 Bass kernel for nn_BasicDeconvolutionBlock (two-phase design).

Reference computation:
    gathered = feats[in_map]                         # [K, M, Cin]
    contrib  = einsum('kmc,kcd->kmd', gathered, W)   # [K, M, Cout]
    out      = zeros([n_out, Cout]).at[out_map].add(contrib)
    y        = relu(batchnorm(out))                  # batch stats over n_out rows

Strategy (8 NeuronCores, SPMD, output-row sharding):
  Host routes each (k, m) pair to the core owning its output row
  (row blocks of n_out/8, ~169k pairs/core), orders the pairs by
  (feats-chunk, k, out-tile) with per-(chunk,k,out-tile) "cells" padded to
  even length, groups (chunk,k) padded to 128.

  Phase A (gather-GEMM): SWDGE dma_gather (transpose) pulls feats rows
  (fp16, 256B) channel-major; per-128-slot matmul against W[k] (fp16);
  PSUM -> fp16 slab (scalar engine Copy) -> contiguous HBM contrib table
  (slot-major 128B rows, in window tensors of 64k slots so phase B's
  int16 gather indices stay in range).  No scatter-add, no occurrence
  rounds.

  Phase B (gather-reduce): contrib rows are fetched in out-tile order as
  PAIRS (256B descriptors = 2 rows, halving descriptor count; cells are
  even-aligned so pairs never straddle cells).  One-hot S matrices
  ([128 pairs x 128 rows], fp16) are built on-chip with a single
  broadcast is_equal against an iota tile per gather call; matmul
  lhsT=S, rhs=gathered pair block accumulates the segmented scatter-add
  directly in PSUM per 128-row out-tile.  Tiles accumulate across the
  window passes into an SBUF fp32 slab.

  BN: per-tile ones-matmul row sums + sum of squares, [2,64] AllReduce
  across 8 cores, batched normalize + ReLU, output shard [25088,64] fp32.
"""

import numpy as np

BN_EPS = 1e-5
SEG = 896            # max descriptors per SWDGE call (Q7 ucode ring limit)
CHUNK = 32768        # int16 gather index range per feats chunk
WSLOTS = 65536       # contrib-table window: 32768 pairs of slots


def _lazy():
    global F32, F16, I16, mybir, bacc, tile
    import sys
    for p in ("/opt/trn_rl_repo",):
        if p not in sys.path:
            sys.path.insert(0, p)
    from concourse import bacc as _bacc, mybir as _mybir
    import concourse.tile as _tile
    mybir, bacc, tile = _mybir, _bacc, _tile
    F32 = mybir.dt.float32
    F16 = mybir.dt.float16
    I16 = mybir.dt.int16


def _roundup(x, m):
    return (x + m - 1) // m * m


def _plan(in_map, out_map, n_out, n_cores, rows_per_core, chunk, wslots):
    """Host-side routing. Returns a dict plan + per-core packed arrays."""
    K, M = in_map.shape
    tiles = _roundup(rows_per_core, 128) // 128
    wpairs = wslots // 2
    in_flat = np.asarray(in_map).ravel().astype(np.int64)
    out_flat = np.asarray(out_map).ravel().astype(np.int64)
    k_idx = np.repeat(np.arange(K, dtype=np.int64), M)
    core = out_flat // rows_per_core
    row_local = out_flat - core * rows_per_core
    t_idx = row_local >> 7
    r128 = row_local & 127
    chnk = in_flat // chunk
    idx_local = in_flat - chnk * chunk
    nchunk = int(chnk.max()) + 1
    NG = nchunk * K
    NCELLS = NG * tiles
    cell = (chnk * K + k_idx) * tiles + t_idx

    sizes = np.zeros((n_cores, NCELLS), np.int64)
    for c in range(n_cores):
        sizes[c] = np.bincount(cell[core == c], minlength=NCELLS)
    ce = (sizes + 1) // 2 * 2                       # cell sizes even-padded
    gsz = ce.reshape(n_cores, NG, tiles).sum(-1)
    gcap = _roundup(gsz.max(0), 128)                # [NG] shared
    gstart = np.zeros(NG + 1, np.int64)
    gstart[1:] = np.cumsum(gcap)
    AS = int(gstart[-1])                            # total A slots

    # ---- A-table layout units ----
    # vertical 8-tile units: table_pos = ubase + p*8 + t (1KB write runs);
    # horizontal remainder units (<8 tiles): table_pos = ubase + t*128 + p.
    k_of_group = np.tile(np.arange(K, dtype=np.int64), nchunk)
    units = []   # (ubase, nt, vertical, k, chunk_id)
    for g in range(NG):
        tiles_g = int(gcap[g]) // 128
        ub = int(gstart[g])
        for u in range(tiles_g // 8):
            units.append([ub + u * 1024, 8, True,
                          int(k_of_group[g]), g // K])
        rem = tiles_g % 8
        if rem:
            units.append([ub + (tiles_g // 8) * 1024, rem, False,
                          int(k_of_group[g]), g // K])
    ubounds = np.array([u[0] for u in units] + [AS], np.int64)

    # windows cut at unit boundaries, each <= wslots slots
    wb = [0]
    while wb[-1] < AS:
        i = int(np.searchsorted(ubounds, wb[-1] + wslots, side="right")) - 1
        nxt = int(ubounds[i]) if int(ubounds[i]) > wb[-1] else AS
        nxt = min(nxt, AS)
        assert nxt > wb[-1]
        wb.append(nxt)
    if len(wb) >= 2 and wb[-1] - wb[-2] >= 8192:
        lo_, hi_ = wb[-2], wb[-1]
        mid = lo_ + (hi_ - lo_) // 2
        i = int(np.searchsorted(ubounds, mid, side="right")) - 1
        cut = int(ubounds[i])
        if lo_ < cut < hi_:
            wb.insert(len(wb) - 1, cut)
    nwin = len(wb) - 1
    wb = np.array(wb, np.int64)
    wbp = wb // 2                                    # pair-space bounds
    for u in units:
        u.append(int(np.searchsorted(wb, u[0], side="right")) - 1)

    ce3 = ce.reshape(n_cores, NG, tiles)
    cstart = (np.cumsum(ce3, axis=2) - ce3
              + gstart[None, :NG, None])            # [cores, NG, tiles]
    cstart_f = cstart.reshape(n_cores, NCELLS)

    order = np.lexsort((cell, core))
    cell_s, core_s = cell[order], core[order]
    key = core_s * NCELLS + cell_s
    n = len(key)
    first = np.ones(n, bool)
    first[1:] = key[1:] != key[:-1]
    gs = np.maximum.accumulate(np.where(first, np.arange(n), 0))
    pos = np.arange(n) - gs
    Apos_s = cstart_f[core_s, cell_s] + pos

    gidxA = np.zeros((n_cores, AS), np.int16)
    rowsA = np.full((n_cores, AS), 255, np.int16)
    gidxA[core_s, Apos_s] = idx_local[order].astype(np.int16)
    rowsA[core_s, Apos_s] = r128[order].astype(np.int16)

    # ---- B stream ----
    p0 = cstart_f // 2
    cnt2 = ce.reshape(n_cores, NCELLS) // 2          # pairs per cell
    t_of_cell = np.tile(np.arange(tiles, dtype=np.int64), NG)

    sizes_B = np.zeros((n_cores, nwin, tiles), np.int64)
    for c in range(n_cores):
        for w in range(nwin):
            lo, hi = int(wbp[w]), int(wbp[w + 1])
            ov = np.clip(np.minimum(p0[c] + cnt2[c], hi)
                         - np.maximum(p0[c], lo), 0, None)
            sizes_B[c, w] = ov.reshape(NG, tiles).sum(0)
    CB = sizes_B.max(0)                              # [nwin, tiles] shared
    sec = CB.sum(1)
    secpad = _roundup(sec, 128)
    wstart = np.zeros(nwin + 1, np.int64)
    wstart[1:] = np.cumsum(secpad)
    BS = int(wstart[-1])                             # total B pairs
    P_wt = np.cumsum(CB, axis=1) - CB + wstart[:nwin, None]

    bidx = np.zeros((n_cores, BS), np.int16)
    brow = np.full((n_cores, BS, 2), 255, np.int16)
    for c in range(n_cores):
        tot = int(cnt2[c].sum())
        if tot == 0:
            continue
        cums = np.cumsum(cnt2[c]) - cnt2[c]
        ap_all = (np.repeat(p0[c], cnt2[c])
                  + np.arange(tot) - np.repeat(cums, cnt2[c]))
        t_all = np.repeat(t_of_cell, cnt2[c])
        w_all = np.searchsorted(wbp, ap_all, side="right") - 1
        key2 = w_all * tiles + t_all
        o2 = np.lexsort((ap_all, key2))
        k2, a2, t2, w2 = key2[o2], ap_all[o2], t_all[o2], w_all[o2]
        f2 = np.ones(tot, bool)
        f2[1:] = k2[1:] != k2[:-1]
        gs2 = np.maximum.accumulate(np.where(f2, np.arange(tot), 0))
        pos2 = np.arange(tot) - gs2
        bpos = P_wt[w2, t2] + pos2
        bidx[c, bpos] = (a2 - wbp[w2]).astype(np.int16)
        brow[c, bpos, 0] = rowsA[c, a2 * 2]
        brow[c, bpos, 1] = rowsA[c, a2 * 2 + 1]

    # entries (block, w, t) + per-(w,t) entry spans
    entries = []
    ent_span = {}
    for w in range(nwin):
        for t in range(tiles):
            if CB[w, t] == 0:
                continue
            b0 = int(P_wt[w, t]) // 128
            b1 = (int(P_wt[w, t]) + int(CB[w, t]) + 127) // 128
            ent_span[(w, t)] = (len(entries), len(entries) + b1 - b0)
            entries.extend((b, w, t) for b in range(b0, b1))
    NENT = len(entries)

    metaI = np.full((n_cores, NENT, 128, 2), 255, np.int16)
    for e, (b, w, t) in enumerate(entries):
        lo = max(b * 128, int(P_wt[w, t]))
        hi = min((b + 1) * 128, int(P_wt[w, t]) + int(CB[w, t]))
        if hi > lo:
            metaI[:, e, lo - b * 128:hi - b * 128, :] = brow[:, lo:hi, :]
    meta = metaI.transpose(0, 2, 1, 3).astype(np.float16)

    # ---- segmentation ----
    # A calls: cut at chunk-section and window boundaries, then SEG slots.
    csec = [int(gstart[ci * K]) for ci in range(nchunk + 1)]
    bounds = sorted(set(csec + wb.tolist()))
    uslot = np.array([u[0] for u in units], np.int64)
    a_calls = []   # (slot0, ns, chunk_id, win, [(k, unit_id, t_local)])
    for lo, hi in zip(bounds[:-1], bounds[1:]):
        s = lo
        while s < hi:
            ns = min(SEG, hi - s)
            tinfo = []
            for j in range(ns // 128):
                sj = s + 128 * j
                uid = int(np.searchsorted(uslot, sj, side="right")) - 1
                tinfo.append((units[uid][3], uid,
                              (sj - units[uid][0]) // 128))
            a_calls.append(
                (s, ns, int(np.searchsorted(csec, s, side="right") - 1),
                 int(np.searchsorted(wb, s, side="right")) - 1, tinfo))
            s += ns

    # B calls: per window section, SEG-pair chunks (128-multiples)
    b_calls = []   # (pair0, np_, w)
    for w in range(nwin):
        s = int(wstart[w])
        hi = int(wstart[w + 1])
        while s < hi:
            np_ = min(SEG, hi - s)
            b_calls.append((s, np_, w))
            s += np_
    call_lo = np.array([c[0] for c in b_calls])
    call_ent = [[] for _ in b_calls]
    for e, (b, w, t) in enumerate(entries):
        ci = int(np.searchsorted(call_lo, b * 128, side="right") - 1)
        assert b_calls[ci][0] <= b * 128 < b_calls[ci][0] + b_calls[ci][1]
        call_ent[ci].append(e)
    ne_max = max((len(x) for x in call_ent), default=0)

    first_w = {}
    last_w = {}
    for t in range(tiles):
        for w in range(nwin):
            if CB[w, t] > 0:
                if t not in first_w:
                    first_w[t] = w
                last_w[t] = w

    table_of_stream = np.empty(AS, np.int64)
    for (ub, nt, vert, _k, _ch, _w) in units:
        n_ = nt * 128
        if vert:
            q = (np.arange(128)[None, :] * nt
                 + np.arange(nt)[:, None])           # [t, p] -> table off
            table_of_stream[ub:ub + n_] = ub + q.reshape(-1)
        else:
            table_of_stream[ub:ub + n_] = ub + np.arange(n_)

    gidxA_w = np.zeros((n_cores, 128, AS // 16), np.int16)
    gidxB_w = np.zeros((n_cores, 128, BS // 16), np.int16)
    for c in range(n_cores):
        gidxA_w[c] = np.tile(
            gidxA[c][table_of_stream].reshape(-1, 16).T, (8, 1))
        gidxB_w[c] = np.tile(bidx[c].reshape(-1, 16).T, (8, 1))

    plan = dict(
        K=K, tiles=tiles, nchunk=nchunk, nwin=nwin, AS=AS, BS=BS,
        chunk=chunk, wslots=wslots, rows_per_core=rows_per_core,
        a_calls=a_calls, b_calls=b_calls, entries=entries,
        ent_span=ent_span, call_ent=call_ent, ne_max=ne_max,
        first_w=first_w, last_w=last_w, NENT=NENT, n_out=int(n_out),
        units=units, wb=wb.tolist(), table_of_stream=table_of_stream,
    )
    arrays = dict(gidxA=gidxA_w, gidxB=gidxB_w, meta=meta)
    return plan, arrays


def _build(plan, n_cores, ftab_rows):
    """Trace the Bass program."""
    _lazy()
    nc = bacc.Bacc("TRN2", target_bir_lowering=False, debug=False)

    K, tiles, nwin = plan["K"], plan["tiles"], plan["nwin"]
    AS, BS, NENT = plan["AS"], plan["BS"], plan["NENT"]
    ne_max = max(plan["ne_max"], 1)
    wslots = plan["wslots"]
    chunk = plan["chunk"]
    n_out = plan["n_out"]
    Cout = 64
    rows_pad = tiles * 128

    ftab = nc.dram_tensor("ftab", [ftab_rows, 128], F16, kind="ExternalInput")
    wt = nc.dram_tensor("wt", [128, K * Cout], F16, kind="ExternalInput")
    gidxA = nc.dram_tensor("gidxA", [128, AS // 16], I16, kind="ExternalInput")
    gidxB = nc.dram_tensor("gidxB", [128, BS // 16], I16, kind="ExternalInput")
    meta = nc.dram_tensor("meta", [128, NENT, 2], F16, kind="ExternalInput")
    iota2 = nc.dram_tensor("iota2", [128, 128, 2], F16,
                           kind="ExternalInput")
    gb = nc.dram_tensor("gb", [2, Cout], F32, kind="ExternalInput")
    atabs = [nc.dram_tensor(f"atab{w}", [wslots // 2, 128], F16)
             for w in range(nwin)]
    cc_in = nc.dram_tensor("cc_in", [2, Cout], F32)
    cc_out = nc.dram_tensor("cc_out", [2, Cout], F32, addr_space="Shared")
    y = nc.dram_tensor("y", [rows_pad, Cout], F16, kind="ExternalOutput")

    units = plan["units"]
    wb = plan["wb"]
    entries = plan["entries"]
    ent_span = plan["ent_span"]
    call_ent = plan["call_ent"]
    first_w = plan["first_w"]
    last_w = plan["last_w"]
    # full 4-tile groups: every (w, t) cell must be populated
    assert tiles % 4 == 0
    for w in range(nwin):
        for t in range(tiles):
            assert (w, t) in ent_span, (w, t)
    assert all(v == 0 for v in first_w.values())
    assert all(v == nwin - 1 for v in last_w.values())
    tg_stat_first = 0
    tg_stat_last = tiles // 4 - 1
    ent_first = {}
    ent_last = {}
    for (w, t), (e0, e1) in ent_span.items():
        ent_first[(w, t)] = e0
        ent_last[(w, t)] = e1 - 1

    with tile.TileContext(nc) as tc:
        with (
            tc.tile_pool(name="const", bufs=1) as cpool,
            tc.tile_pool(name="agix", bufs=3) as agix,
            tc.tile_pool(name="ag", bufs=3) as agp,
            tc.tile_pool(name="aslab", bufs=3) as aslab,
            tc.tile_pool(name="bgix", bufs=3) as bgix,
            tc.tile_pool(name="bg", bufs=3) as bgp,
            tc.tile_pool(name="bmeta", bufs=3) as bmeta,
            tc.tile_pool(name="bs", bufs=3) as bspool,
            tc.tile_pool(name="slab", bufs=1) as slabpool,
        ):
            w_sb = cpool.tile([128, K * Cout], F16, tag="w")
            nc.sync.dma_start(out=w_sb[:, :], in_=wt[:, :])
            iota_sb = cpool.tile([128, 128, 2], F16, tag="iota")
            nc.sync.dma_start(out=iota_sb[:, :, :], in_=iota2[:, :, :])
            out_slab = slabpool.tile([128, tiles, Cout], F32, tag="slab")

            a_by_w = [[] for _ in range(nwin)]
            for call in plan["a_calls"]:
                a_by_w[call[3]].append(call)
            b_by_w = [[] for _ in range(nwin)]
            for ci, call in enumerate(plan["b_calls"]):
                b_by_w[call[2]].append((ci, call))

            psum_of = {}
            psum_of_u = {}
            apsum, bpsum = [], []

            ABATCH = 8

            def a_batches(w):
                calls = a_by_w[w]
                outs = []
                for i0 in range(0, len(calls), ABATCH):
                    outs.append(_mk_a(calls[i0:i0 + ABATCH], w, i0))
                return outs

            def _mk_a(batch, w, i0):
                def go():
                    bs0 = batch[0][0]
                    bs1 = batch[-1][0] + batch[-1][1]
                    gib = agix.tile([128, ABATCH * SEG // 16], I16, tag="agi")
                    ldq = nc.scalar if (i0 // ABATCH) % 2 else nc.sync
                    ldq.dma_start(
                        out=gib[:, :(bs1 - bs0) // 16],
                        in_=gidxA[:, bs0 // 16:bs1 // 16])
                    for (s0, ns, ch, _w, tinfo) in batch:
                        g = agp.tile([128, 1, SEG], F16, tag="ag")
                        nc.gpsimd.dma_gather(
                            out_ap=g[:, :, :ns],
                            in_ap=ftab[ch * chunk:(ch + 1) * chunk, :],
                            idxs_ap=gib[:, (s0 - bs0) // 16:
                                        (s0 - bs0 + ns) // 16],
                            num_idxs=ns,
                            num_idxs_reg=ns,
                            elem_size=128,
                            transpose=True,
                        )
                        for j, (k, uid, tl) in enumerate(tinfo):
                            if uid not in psum_of_u:
                                psum_of_u[uid] = apsum[0].tile(
                                    [128, 8, Cout], F32, tag="aps",
                                    name=f"aps_{uid}")
                            psu = psum_of_u[uid]
                            nc.tensor.matmul(
                                out=psu[:, tl, :],
                                lhsT=g[:, 0, j * 128:(j + 1) * 128],
                                rhs=w_sb[:, k * Cout:(k + 1) * Cout],
                                start=True, stop=True,
                            )
                            ub, ntu, vert, _k2, _c2, uw = units[uid]
                            if tl == ntu - 1:
                                sl = aslab.tile(
                                    [128, 8, Cout], F16, tag="asl",
                                    name=f"asl_{uid}")
                                nc.scalar.activation(
                                    out=sl[:, :ntu, :], in_=psu[:, :ntu, :],
                                    func=mybir.ActivationFunctionType.Copy)
                                be = (ub - wb[uw]) * 64
                                ne_ = ntu * 128 * 64
                                flat = atabs[uw][:, :].flatten()
                                if vert:
                                    oap = flat[be:be + ne_].rearrange(
                                        "(p t c) -> p t c", t=ntu, c=64)
                                else:
                                    oap = flat[be:be + ne_].rearrange(
                                        "(t p c) -> p t c", p=128, c=64)
                                wq = (nc.sync if (s0 // SEG) % 2
                                      else nc.scalar)
                                wq.dma_start(out=oap, in_=sl[:, :ntu, :])
                                del psum_of_u[uid]
                return go

            BBATCH = 8

            def b_batches(w):
                calls = b_by_w[w]
                outs = []
                for i0 in range(0, len(calls), BBATCH):
                    outs.append(_mk_b(calls[i0:i0 + BBATCH], w, i0))
                return outs

            def _mk_b(bat, w, i0):
                def go():
                    bp0 = bat[0][1][0]
                    bp1 = bat[-1][1][0] + bat[-1][1][1]
                    gib = bgix.tile([128, BBATCH * SEG // 16], I16, tag="bgi")
                    ldq = nc.scalar if (i0 // BBATCH) % 2 else nc.sync
                    ldq.dma_start(
                        out=gib[:, :(bp1 - bp0) // 16],
                        in_=gidxB[:, bp0 // 16:bp1 // 16])
                    be_lo = call_ent[bat[0][0]][0]
                    be_hi = call_ent[bat[-1][0]][-1] + 1
                    mtb = bmeta.tile([128, BBATCH * ne_max, 2], F16, tag="bm")
                    nc.scalar.dma_start(
                        out=mtb[:, :be_hi - be_lo, :],
                        in_=meta[:, be_lo:be_hi, :])
                    for (ci, (pair0, np_, _w)) in bat:
                        nb = np_ // 128
                        g = bgp.tile([128, SEG // 128, 128], F16, tag="bg")
                        nc.gpsimd.dma_gather(
                            out_ap=g[:, :nb, :],
                            in_ap=atabs[w][:, :],
                            idxs_ap=gib[:, (pair0 - bp0) // 16:
                                        (pair0 - bp0 + np_) // 16],
                            num_idxs=np_,
                            num_idxs_reg=np_,
                            elem_size=128,
                        )
                        ents = call_ent[ci]
                        if not ents:
                            continue
                        ne = len(ents)
                        e_lo = ents[0]
                        st = bspool.tile(
                            [128, 128, ne_max, 2], F16, tag="bsl")
                        nc.vector.tensor_tensor(
                            out=st[:, :, :ne, :],
                            in0=mtb[:, e_lo - be_lo:e_lo - be_lo + ne, :]
                            .unsqueeze(1).broadcast_to([128, 128, ne, 2]),
                            in1=iota_sb[:, :, :].unsqueeze(2)
                            .broadcast_to([128, 128, ne, 2]),
                            op=mybir.AluOpType.is_equal,
                        )
                        for e in ents:
                            b, we, t = entries[e]
                            tg = t // 4
                            if (we, tg) not in psum_of:
                                psum_of[(we, tg)] = bpsum[0].tile(
                                    [128, 4, Cout], F32, tag="bps",
                                    name=f"bps_{we}_{tg}")
                            ps = psum_of[(we, tg)]
                            last = e == ent_last[(we, t)]
                            for eo in range(2):
                                nc.tensor.matmul(
                                    out=ps[:, t % 4, :],
                                    lhsT=st[:, :, e - e_lo, eo],
                                    rhs=g[:, b - pair0 // 128,
                                          eo * 64:(eo + 1) * 64],
                                    start=(e == ent_first[(we, t)]
                                           and eo == 0),
                                    stop=(last and eo == 1),
                                )
                            if last and t == tg * 4 + 3:
                                sl_ = out_slab[:, tg * 4:tg * 4 + 4, :]
                                if we == 0:
                                    nc.vector.tensor_copy(
                                        out=sl_, in_=ps[:, :, :])
                                else:
                                    nc.vector.tensor_tensor(
                                        out=sl_, in0=sl_, in1=ps[:, :, :],
                                        op=mybir.AluOpType.add)
                                del psum_of[(we, tg)]
                                if we == nwin - 1:
                                    sqt = aslab.tile(
                                        [128, 4, Cout], F32, tag="sq",
                                        name=f"sq_{tg}")
                                    nc.vector.tensor_tensor(
                                        out=sqt[:, :, :],
                                        in0=sl_, in1=sl_,
                                        op=mybir.AluOpType.mult)
                                    nc.tensor.matmul(
                                        out=stat_ps[:, 0:256],
                                        lhsT=ones1[:, :],
                                        rhs=sl_,
                                        start=(tg == tg_stat_first),
                                        stop=(tg == tg_stat_last))
                                    nc.tensor.matmul(
                                        out=stat_ps[:, 256:512],
                                        lhsT=ones1[:, :],
                                        rhs=sqt[:, :, :],
                                        start=(tg == tg_stat_first),
                                        stop=(tg == tg_stat_last))
                return go

            with (
                tc.tile_pool(name="apsum", bufs=3, space="PSUM") as apsum_,
                tc.tile_pool(name="bpsum", bufs=4, space="PSUM") as bpsum_,
                tc.tile_pool(name="spsum", bufs=1, space="PSUM") as spsum_,
            ):
                apsum.append(apsum_)
                bpsum.append(bpsum_)
                stat_ps = spsum_.tile([1, 512], F32, tag="stat")
                ones1 = cpool.tile([128, 1], F32, tag="ones1")
                nc.vector.memset(ones1[:, :], 1.0)
                for go in a_batches(0):
                    go()
                for w in range(1, nwin):
                    A, B = a_batches(w), b_batches(w - 1)
                    na, nb = len(A), len(B)
                    ia = ib = 0
                    while ia < na or ib < nb:
                        if ia < na and (ib >= nb or ia * nb <= ib * na):
                            A[ia]()
                            ia += 1
                        else:
                            B[ib]()
                            ib += 1
                for go in b_batches(nwin - 1):
                    go()
                stf = cpool.tile([1, 512], F32, tag="stf")
                nc.vector.tensor_copy(out=stf[:, :], in_=stat_ps[:, :])
                # fold 4 tile-columns into one (sum region and sq region)
                st0 = cpool.tile([1, Cout], F32, tag="st0")
                st1 = cpool.tile([1, Cout], F32, tag="st1")
                nc.vector.tensor_copy(out=st0[:, :], in_=stf[:, 0:64])
                for jj in range(1, 4):
                    nc.vector.tensor_tensor(
                        out=st0[:, :], in0=st0[:, :],
                        in1=stf[:, jj * 64:(jj + 1) * 64],
                        op=mybir.AluOpType.add)
                nc.vector.tensor_copy(out=st1[:, :], in_=stf[:, 256:320])
                for jj in range(1, 4):
                    nc.vector.tensor_tensor(
                        out=st1[:, :], in0=st1[:, :],
                        in1=stf[:, 256 + jj * 64:256 + (jj + 1) * 64],
                        op=mybir.AluOpType.add)
                nc.sync.dma_start(out=cc_in[0:1, :], in_=st0[:, :])
                nc.sync.dma_start(out=cc_in[1:2, :], in_=st1[:, :])
                nc.gpsimd.collective_compute(
                    "AllReduce",
                    mybir.AluOpType.add,
                    ins=[cc_in[:, :]],
                    outs=[cc_out[:, :]],
                    replica_groups=[list(range(n_cores))],
                )

            # ---- BN + ReLU ----
            with (
                tc.tile_pool(name="bn", bufs=4) as bnp,
                tc.tile_pool(name="bnps", bufs=2, space="PSUM") as bnps,
            ):
                gs0 = bnp.tile([1, Cout], F32, tag="gs0")
                gs1 = bnp.tile([1, Cout], F32, tag="gs1")
                nc.sync.dma_start(out=gs0[:, :], in_=cc_out[0:1, :])
                nc.sync.dma_start(out=gs1[:, :], in_=cc_out[1:2, :])
                gam_t = bnp.tile([1, Cout], F32, tag="gam")
                bet_t = bnp.tile([1, Cout], F32, tag="bet")
                nc.sync.dma_start(out=gam_t[:, :], in_=gb[0:1, :])
                nc.sync.dma_start(out=bet_t[:, :], in_=gb[1:2, :])

                inv_n = 1.0 / float(n_out)
                mean_t = bnp.tile([1, Cout], F32, tag="mean")
                ex2_t = bnp.tile([1, Cout], F32, tag="ex2")
                var_t = bnp.tile([1, Cout], F32, tag="var")
                sd_t = bnp.tile([1, Cout], F32, tag="sd")
                rs_t = bnp.tile([1, Cout], F32, tag="rs")
                a_t = bnp.tile([1, Cout], F32, tag="a")
                b_t = bnp.tile([1, Cout], F32, tag="b")
                nc.vector.tensor_scalar_mul(mean_t[:, :], gs0[:, :], inv_n)
                nc.vector.tensor_scalar_mul(ex2_t[:, :], gs1[:, :], inv_n)
                nc.vector.tensor_tensor(
                    out=var_t[:, :], in0=mean_t[:, :], in1=mean_t[:, :],
                    op=mybir.AluOpType.mult)
                nc.vector.tensor_tensor(
                    out=var_t[:, :], in0=ex2_t[:, :], in1=var_t[:, :],
                    op=mybir.AluOpType.subtract)
                nc.vector.tensor_scalar_add(var_t[:, :], var_t[:, :], BN_EPS)
                nc.scalar.activation(
                    out=sd_t[:, :], in_=var_t[:, :],
                    func=mybir.ActivationFunctionType.Sqrt)
                nc.vector.reciprocal(out=rs_t[:, :], in_=sd_t[:, :])
                nc.vector.tensor_tensor(
                    out=a_t[:, :], in0=gam_t[:, :], in1=rs_t[:, :],
                    op=mybir.AluOpType.mult)
                nc.vector.tensor_tensor(
                    out=b_t[:, :], in0=mean_t[:, :], in1=a_t[:, :],
                    op=mybir.AluOpType.mult)
                nc.vector.tensor_tensor(
                    out=b_t[:, :], in0=bet_t[:, :], in1=b_t[:, :],
                    op=mybir.AluOpType.subtract)
                ones_row = bnp.tile([1, 128], F32, tag="ones_row")
                nc.vector.memset(ones_row[:, :], 1.0)
                a_full = bnp.tile([128, Cout], F32, tag="afull")
                b_full = bnp.tile([128, Cout], F32, tag="bfull")
                ab_ps = bnps.tile([128, Cout], F32, tag="abps")
                nc.tensor.matmul(
                    out=ab_ps[:, :], lhsT=ones_row[:, :], rhs=a_t[:, :],
                    start=True, stop=True)
                nc.vector.tensor_copy(out=a_full[:, :], in_=ab_ps[:, :])
                nc.tensor.matmul(
                    out=ab_ps[:, :], lhsT=ones_row[:, :], rhs=b_t[:, :],
                    start=True, stop=True)
                nc.vector.tensor_copy(out=b_full[:, :], in_=ab_ps[:, :])
                # batched normalize + relu + write, pipelined in chunks
                yv = y[:, :].flatten().rearrange(
                    "(t p c) -> p t c", p=128, c=64)
                nchk = 4
                step = tiles // nchk
                for ci_ in range(nchk):
                    tlo = ci_ * step
                    thi = tiles if ci_ == nchk - 1 else (ci_ + 1) * step
                    nn = thi - tlo
                    sl_ = out_slab[:, tlo:thi, :]
                    nc.vector.tensor_tensor(
                        out=sl_, in0=sl_,
                        in1=a_full[:, :].unsqueeze(1)
                        .broadcast_to([128, nn, Cout]),
                        op=mybir.AluOpType.mult)
                    nc.vector.tensor_tensor(
                        out=sl_, in0=sl_,
                        in1=b_full[:, :].unsqueeze(1)
                        .broadcast_to([128, nn, Cout]),
                        op=mybir.AluOpType.add)
                    y16 = bnp.tile([128, step, Cout], F16, tag="y16")
                    nc.scalar.activation(
                        out=y16[:, :nn, :], in_=sl_,
                        func=mybir.ActivationFunctionType.Relu)
                    wq = nc.sync if ci_ % 2 else nc.scalar
                    wq.dma_start(out=yv[:, tlo:thi, :], in_=y16[:, :nn, :])

    nc.compile()
    return nc


def _prepare(feats, W, gamma, beta, in_map, out_map, n_out,
             n_cores=8, dup_safe=False, expand=1):
    """Host prep shared by kernel() and tests. Returns (nc, in_maps, plan)."""
    _lazy()
    n_out = int(n_out)
    K, Cin, Cout = W.shape
    assert Cin == 64 and Cout == 64
    rows_per_core = n_out // n_cores
    assert rows_per_core * n_cores == n_out

    in_map = np.asarray(in_map, dtype=np.int64)
    out_map = np.asarray(out_map, dtype=np.int64)
    feats = np.asarray(feats, dtype=np.float32)
    W = np.asarray(W, dtype=np.float32)

    plan, arrays = _plan(in_map, out_map, n_out, n_cores, rows_per_core,
                         CHUNK, WSLOTS)

    ftab_rows = _roundup(feats.shape[0], CHUNK)
    ftab = np.zeros((ftab_rows, 128), dtype=np.float16)
    ftab[:feats.shape[0], :64] = feats.astype(np.float16)
    wt = np.zeros((128, K * 64), dtype=np.float16)
    wt[:64, :] = W.transpose(1, 0, 2).reshape(64, K * 64).astype(np.float16)
    gb = np.stack([np.asarray(gamma, np.float32),
                   np.asarray(beta, np.float32)])
    iota2 = np.broadcast_to(np.arange(128, dtype=np.float32)[None, :, None],
                            (128, 128, 2)).astype(np.float16)

    nc = _build(plan, n_cores, ftab_rows)
    in_maps = [
        dict(ftab=ftab, wt=wt, gb=gb, iota2=iota2,
             gidxA=arrays["gidxA"][c], gidxB=arrays["gidxB"][c],
             meta=arrays["meta"][c])
        for c in range(n_cores)
    ]
    return nc, in_maps, plan


def kernel(feats, W, gamma, beta, in_map, out_map, n_out):
    _lazy()
    from concourse.bass_utils import run_bass_kernel_spmd

    n_cores = 8
    nc, in_maps, plan = _prepare(
        feats, W, gamma, beta, in_map, out_map, n_out, n_cores)
    res = run_bass_kernel_spmd(nc, in_maps, list(range(n_cores)))
    rows = plan["rows_per_core"]
    out = np.concatenate(
        [res.results[c]["y"][:rows] for c in range(n_cores)], axis=0)
    return out.astype(np.float32)
